# revision 5
# baseline (speedup 1.0000x reference)
"""GAT Trainium kernel: host preprocessing + bass program builder.

Design (dst-range sharding across C cores):
- Core c owns dst nodes [c*NL, (c+1)*NL). Edges partitioned by dst.
- Edges sorted by (src_chunk, dst_loc); src ids chunked into groups of <=32767
  rows so dma_gather int16 indices work.
- Per (block b of 128 dst, chunk ch): edge run padded to a multiple of 128 with
  a GLOBAL (cross-core max) tile count T[b][ch] -> fully uniform SPMD code.
  All per-core variation lives in data (gather indices + one-hot S matrices).
- Edge phase per batch: dma_gather G rows (768B: h(128)|el(4)|pad) by src;
  dma_gather er (4 of 64B span) from local slab by dst; ex=exp(leaky(el+er))
  in place; Hw=ex*h in place; psum[128,132] += S_tile.T @ [Hw|ex] per (b,ch);
  psum accumulated into SBUF A[128, NB*132] across chunks.
- Node phase per block: h' = relu(A_u / max(A_s,eps) + b); transpose; matmul
  W_next_ext -> next layer G rows [h'|el'|er'|pad]; AllGather slabs.
- Layer 3 (1 head, 40 feats): 256B G rows; ends with log_softmax.
"""
from contextlib import ExitStack
import numpy as np
import concourse.bass as bass
import concourse.tile as tile
from concourse import bacc, mybir
from concourse.masks import make_identity

F32 = mybir.dt.float32
I16 = mybir.dt.int16

IN, HID, HEADS, OUT = 256, 32, 4, 40
HH = HID * HEADS  # 128
NEG_SLOPE = 0.2
BT = 16  # tiles per gather batch (16*128 = 2048 idx/call)


def wrap16(a):
    n = a.shape[0]
    assert n % 16 == 0
    blk = a.reshape(-1, 16).T  # [16, n/16]
    return np.tile(blk, (8, 1)).astype(np.int16)


def host_preprocess(src, dst, n_nodes, n_cores=8, n_chunks=4):
    NL = n_nodes // n_cores
    assert NL * n_cores == n_nodes
    NB = (NL + 127) // 128
    CS = (n_nodes + n_chunks - 1) // n_chunks
    assert CS <= 32767

    src = np.asarray(src); dst = np.asarray(dst)
    core_of = dst // NL
    per_core = []
    counts = np.zeros((n_cores, NB, n_chunks), np.int64)
    for c in range(n_cores):
        m = core_of == c
        s, d = src[m], dst[m]
        dloc = d - c * NL
        ch = s // CS
        sloc = s % CS
        order = np.lexsort((dloc, ch))
        dloc, ch, sloc = dloc[order], ch[order], sloc[order]
        b = dloc // 128
        per_core.append((sloc, dloc, ch, b))
        for chh in range(n_chunks):
            mm = ch == chh
            bb, cnt = np.unique(b[mm], return_counts=True)
            counts[c, bb, chh] = cnt
    T = np.ceil(counts.max(axis=0) / 128).astype(np.int64)  # [NB, n_chunks]

    segs = []  # chunk-major: (chunk, block, tiles)
    for chh in range(n_chunks):
        for b in range(NB):
            if T[b, chh] > 0:
                segs.append((chh, b, int(T[b, chh])))
    n_tiles = sum(t for _, _, t in segs)
    total_slots = n_tiles * 128

    batches = []
    cur = None
    tglob = 0
    for chh, b, t in segs:
        for _ in range(t):
            if cur is None or cur["chunk"] != chh or cur["nt"] >= BT:
                if cur is not None:
                    batches.append(cur)
                cur = {"chunk": chh, "t0": tglob, "nt": 0}
            cur["nt"] += 1
            tglob += 1
    if cur is not None:
        batches.append(cur)
    assert tglob == n_tiles

    core_data = []
    for c in range(n_cores):
        sloc, dloc, ch, b = per_core[c]
        src16 = np.zeros(total_slots, np.int16)
        dst16 = np.zeros(total_slots, np.int16)
        S = np.zeros((128, n_tiles, 128), np.float32)  # [e, t, d]
        pos = 0
        for chh, bb, t in segs:
            m = (ch == chh) & (b == bb)
            idx = np.nonzero(m)[0]
            n = len(idx)
            cap = t * 128
            assert n <= cap, (c, chh, bb, n, cap)
            sl = sloc[idx]; dl = dloc[idx]
            src16[pos:pos + n] = sl
            dst16[pos:pos + n] = dl
            e_in_seg = np.arange(n)
            tt = pos // 128 + e_in_seg // 128
            ee = e_in_seg % 128
            S[ee, tt, dl - bb * 128] = 1.0
            pos += cap
        assert pos == total_slots
        core_data.append(dict(
            src16=wrap16(src16), dst16=wrap16(dst16),
            S=np.ascontiguousarray(S.reshape(128, n_tiles * 128)),
        ))

    return dict(
        n_cores=n_cores, n_nodes=n_nodes, NL=NL, NB=NB, CS=CS,
        n_chunks=n_chunks, segs=segs, batches=batches, n_tiles=n_tiles,
        total_slots=total_slots, core_data=core_data,
    )


def host_weights(W1, al1, ar1, b1, W2, al2, ar2, b2, W3, al3, ar3, b3):
    def bd(al):
        al = np.asarray(al, np.float32)
        H, F = al.shape
        out = np.zeros((H * F, H), np.float32)
        for h in range(H):
            out[h * F:(h + 1) * F, h] = al[h]
        return out
    W1 = np.asarray(W1, np.float32); W2 = np.asarray(W2, np.float32); W3 = np.asarray(W3, np.float32)
    W1ext = np.concatenate([W1, W1 @ bd(al1), W1 @ bd(ar1)], axis=1)
    W2ext = np.concatenate([W2, W2 @ bd(al2), W2 @ bd(ar2)], axis=1)
    W3ext = np.concatenate([W3, W3 @ bd(al3), W3 @ bd(ar3)], axis=1)
    b1rep = np.tile(np.asarray(b1, np.float32).reshape(1, HH), (128, 1))
    b2rep = np.tile(np.asarray(b2, np.float32).reshape(1, HH), (128, 1))
    b3rep = np.tile(np.asarray(b3, np.float32).reshape(1, OUT), (128, 1))
    return dict(W1ext=W1ext, W2ext=W2ext, W3ext=W3ext,
                b1rep=b1rep, b2rep=b2rep, b3rep=b3rep)


def apx(base_ap, col_off, dims):
    """AP at column offset of a [128, W] tile with custom free dims."""
    b = base_ap[:, col_off:col_off + 1]
    return bass.AP(b.tensor, b.offset, [b.ap[0]] + [list(d) for d in dims])


def build_program(plan, stage=99):
    C = plan["n_cores"]; NL = plan["NL"]; NB = plan["NB"]
    NT = plan["n_tiles"]; TS = plan["total_slots"]
    NLP = NB * 128

    nc = bacc.Bacc("TRN2", target_bir_lowering=False, debug=False, num_devices=C)

    featT = nc.dram_tensor("featT", [IN, NL], F32, kind="ExternalInput").ap()
    W1e = nc.dram_tensor("W1ext", [IN, 136], F32, kind="ExternalInput").ap()
    W2e = nc.dram_tensor("W2ext", [HH, 136], F32, kind="ExternalInput").ap()
    W3e = nc.dram_tensor("W3ext", [HH, 42], F32, kind="ExternalInput").ap()
    B1 = nc.dram_tensor("b1rep", [128, HH], F32, kind="ExternalInput").ap()
    B2 = nc.dram_tensor("b2rep", [128, HH], F32, kind="ExternalInput").ap()
    B3 = nc.dram_tensor("b3rep", [128, OUT], F32, kind="ExternalInput").ap()
    SRC = nc.dram_tensor("src16", [128, TS // 16], I16, kind="ExternalInput").ap()
    DST = nc.dram_tensor("dst16", [128, TS // 16], I16, kind="ExternalInput").ap()
    SM = nc.dram_tensor("S", [128, NT * 128], F32, kind="ExternalInput").ap()
    OUTT = nc.dram_tensor("out", [NLP, OUT], F32, kind="ExternalOutput").ap()

    G1s = nc.dram_tensor("G1slab", [NL, 192], F32).ap()
    G2s = nc.dram_tensor("G2slab", [NL, 192], F32).ap()
    G3s = nc.dram_tensor("G3slab", [NL, 64], F32).ap()
    G1 = nc.dram_tensor("G1", [C * NL, 192], F32, addr_space="Shared").ap()
    G2 = nc.dram_tensor("G2", [C * NL, 192], F32, addr_space="Shared").ap()
    G3 = nc.dram_tensor("G3", [C * NL, 64], F32, addr_space="Shared").ap()

    rg = [list(range(C))]

    def allgather(slab, full):
        if C == 1:
            nc.sync.dma_start(full[:, :], slab[:, :])
        else:
            nc.gpsimd.collective_compute(
                "AllGather", mybir.AluOpType.bypass,
                replica_groups=rg, ins=[slab[:, :]], outs=[full[:, :]])

    with tile.TileContext(nc) as tc, ExitStack() as ctx:
        const = ctx.enter_context(tc.tile_pool(name="const", bufs=1))
        accp = ctx.enter_context(tc.tile_pool(name="acc", bufs=1))

        w1sb = const.tile([128, 2 * 136], F32)
        nc.sync.dma_start(w1sb[:, 0:136], W1e[0:128, :])
        nc.sync.dma_start(w1sb[:, 136:272], W1e[128:256, :])
        w2sb = const.tile([128, 136], F32)
        nc.sync.dma_start(w2sb[:], W2e[:, :])
        w3sb = const.tile([128, 42], F32)
        nc.sync.dma_start(w3sb[:], W3e[:, :])
        b1sb = const.tile([128, HH], F32)
        nc.sync.dma_start(b1sb[:], B1[:, :])
        b2sb = const.tile([128, HH], F32)
        nc.sync.dma_start(b2sb[:], B2[:, :])
        b3sb = const.tile([128, OUT], F32)
        nc.sync.dma_start(b3sb[:], B3[:, :])
        ident = const.tile([128, 128], F32)
        make_identity(nc, ident[:])
        srcsb = const.tile([128, TS // 16], I16)
        nc.sync.dma_start(srcsb[:], SRC[:, :])
        dstsb = const.tile([128, TS // 16], I16)
        nc.sync.dma_start(dstsb[:], DST[:, :])

        # Layer 1 node phase
        with tc.tile_pool(name="l1n", bufs=3) as lp, \
             tc.tile_pool(name="l1np", bufs=2, space="PSUM") as pp:
            for b in range(NB):
                r0 = b * 128
                r1 = min(r0 + 128, NL)
                nr = r1 - r0
                xt = lp.tile([128, 256], F32, tag="xt")
                nc.sync.dma_start(xt[:, 0:nr], featT[0:128, r0:r1])
                nc.sync.dma_start(xt[:, 128:128 + nr], featT[128:256, r0:r1])
                ps = pp.tile([128, 136], F32, tag="ps")
                nc.tensor.matmul(ps[:nr, :], xt[:, 0:nr], w1sb[:, 0:136],
                                 start=True, stop=False)
                nc.tensor.matmul(ps[:nr, :], xt[:, 128:128 + nr], w1sb[:, 136:272],
                                 start=False, stop=True)
                gsb = lp.tile([128, 136], F32, tag="gsb")
                nc.vector.tensor_copy(gsb[:nr, :], ps[:nr, :])
                nc.sync.dma_start(G1s[r0:r1, 0:136], gsb[:nr, :])

        if stage >= 2:
            allgather(G1s, G1)
        if stage >= 3:
            edge_layer(tc, plan, 1, G1, G1s, srcsb, dstsb, SM, accp,
                       w_next=w2sb, b_rep=b1sb, Gn_s=G2s, ident=ident,
                       node_phase=(stage >= 4))
        if stage >= 5:
            allgather(G2s, G2)
            edge_layer(tc, plan, 2, G2, G2s, srcsb, dstsb, SM, accp,
                       w_next=w3sb, b_rep=b2sb, Gn_s=G3s, ident=ident)
            allgather(G3s, G3)
        if stage >= 6:
            edge_layer3(tc, plan, G3, G3s, srcsb, dstsb, SM, accp, b3sb, OUTT)


    nc.compile()
    return nc


def seg_maps(plan):
    seg_of_tile = []
    tile_in_seg = []
    for chh, b, t in plan["segs"]:
        seg_of_tile += [(chh, b, t)] * t
        tile_in_seg += list(range(t))
    return seg_of_tile, tile_in_seg


def edge_layer(tc, plan, lnum, G, Gslab, srcsb, dstsb, SM, accp,
               w_next, b_rep, Gn_s, ident, node_phase=True):
    nc = tc.nc
    NL = plan["NL"]; NB = plan["NB"]; CS = plan["CS"]
    seg_of_tile, tile_in_seg = seg_maps(plan)

    A = accp.tile([128, NB * 132], F32, tag="A")
    nc.gpsimd.memset(A[:], 0.0)

    with tc.tile_pool(name=f"e{lnum}", bufs=3) as ep, \
         tc.tile_pool(name=f"e{lnum}s", bufs=3) as sp, \
         tc.tile_pool(name=f"e{lnum}p", bufs=6, space="PSUM") as pp:
        ps_cur = {}
        for bt in plan["batches"]:
            nt = bt["nt"]; t0 = bt["t0"]; chh = bt["chunk"]
            nidx = nt * 128
            T = ep.tile([128, BT * 192], F32, tag="T")
            Tv = T[:].rearrange("p (k d) -> p k d", d=192)[:, 0:nt, :]
            nc.gpsimd.dma_gather(
                Tv, G[chh * CS:(chh + 1) * CS, :],
                srcsb[:, t0 * 8:(t0 + nt) * 8],
                nidx, nidx, 192, single_packet=False)
            Ter = ep.tile([128, BT * 64], F32, tag="Ter")
            Terv = Ter[:].rearrange("p (k d) -> p k d", d=64)[:, 0:nt, :]
            nc.gpsimd.dma_gather(
                Terv, Gslab[:, 128:192],
                dstsb[:, t0 * 8:(t0 + nt) * 8],
                nidx, nidx, 64, elem_step=192, single_packet=False)
            el = Tv[:, :, 128:132]
            nc.vector.tensor_tensor(out=el, in0=el, in1=Terv[:, :, 4:8],
                                    op=mybir.AluOpType.add)
            lk = ep.tile([128, BT * 4], F32, tag="lk", name="lk")
            lkv = lk[:].rearrange("p (k d) -> p k d", d=4)[:, 0:nt, :]
            nc.vector.tensor_scalar_mul(lkv, el, NEG_SLOPE)
            nc.vector.tensor_tensor(out=el, in0=el, in1=lkv,
                                    op=mybir.AluOpType.max)
            nc.scalar.activation(el, el, mybir.ActivationFunctionType.Exp)
            # Hw = ex * h (views [p, nt, 4, 32])
            h4 = apx(T[:], 0, [[192, nt], [32, 4], [1, 32]])
            ex4 = apx(T[:], 128, [[192, nt], [1, 4], [0, 32]])
            nc.vector.tensor_tensor(out=h4, in0=h4, in1=ex4,
                                    op=mybir.AluOpType.mult)
            Ssb = sp.tile([128, BT * 128], F32, tag="S")
            nc.sync.dma_start(Ssb[:, 0:nt * 128], SM[:, t0 * 128:(t0 + nt) * 128])
            for k in range(nt):
                tg = t0 + k
                _, b, tseg = seg_of_tile[tg]
                tin = tile_in_seg[tg]
                key = b
                if tin == 0:
                    ps_cur[key] = pp.tile([128, 132], F32, tag="ps", name="ps")
                ps = ps_cur[key]
                nc.tensor.matmul(
                    ps[:], Ssb[:, k * 128:(k + 1) * 128],
                    T[:, k * 192:k * 192 + 132],
                    start=(tin == 0), stop=(tin == tseg - 1))
                if tin == tseg - 1:
                    nc.vector.tensor_tensor(
                        out=A[:, b * 132:(b + 1) * 132],
                        in0=A[:, b * 132:(b + 1) * 132],
                        in1=ps[:], op=mybir.AluOpType.add)
                    del ps_cur[key]
        assert not ps_cur

    if not node_phase:
        return
    NOUT = w_next.shape[1]
    with tc.tile_pool(name=f"n{lnum}", bufs=3) as np_, \
         tc.tile_pool(name=f"n{lnum}p", bufs=4, space="PSUM") as pp:
        for b in range(NB):
            r0 = b * 128
            r1 = min(r0 + 128, NL)
            nr = r1 - r0
            Ab = A[:, b * 132:(b + 1) * 132]
            rs = np_.tile([128, 4], F32, tag="rs")
            nc.vector.tensor_scalar_max(rs[:], Ab[:, 128:132], 1e-30)
            nc.vector.reciprocal(rs[:], rs[:])
            hp = np_.tile([128, 128], F32, tag="hp")
            hv = hp[:].rearrange("p (g f) -> p g f", g=4)
            rsb = apx(rs[:], 0, [[1, 4], [0, 32]])
            av = Ab[:, 0:128].rearrange("p (g f) -> p g f", g=4)
            nc.vector.tensor_tensor(out=hv, in0=av, in1=rsb,
                                    op=mybir.AluOpType.mult)
            nc.vector.tensor_tensor(out=hp[:], in0=hp[:], in1=b_rep[:],
                                    op=mybir.AluOpType.add)
            nc.scalar.activation(hp[:], hp[:], mybir.ActivationFunctionType.Relu)
            pst = pp.tile([128, 128], F32, tag="pst")
            nc.tensor.transpose(out=pst[:], in_=hp[:], identity=ident[:])
            hpt = np_.tile([128, 128], F32, tag="hpt")
            nc.vector.tensor_copy(hpt[:], pst[:])
            ps2 = pp.tile([128, NOUT], F32, tag="ps2")
            nc.tensor.matmul(ps2[:nr, :], hpt[:, 0:nr], w_next[:],
                             start=True, stop=True)
            g2sb = np_.tile([128, NOUT], F32, tag="g2sb")
            nc.vector.tensor_copy(g2sb[:nr, :], ps2[:nr, :])
            nc.sync.dma_start(Gn_s[r0:r1, 0:NOUT], g2sb[:nr, :])


def edge_layer3(tc, plan, G, Gslab, srcsb, dstsb, SM, accp, b3sb, OUTT):
    nc = tc.nc
    NL = plan["NL"]; NB = plan["NB"]; CS = plan["CS"]
    seg_of_tile, tile_in_seg = seg_maps(plan)

    A = accp.tile([128, NB * 132], F32, tag="A")
    Av = A[:, 0:NB * 41]
    nc.gpsimd.memset(A[:], 0.0)

    with tc.tile_pool(name="e3", bufs=3) as ep, \
         tc.tile_pool(name="e3s", bufs=3) as sp, \
         tc.tile_pool(name="e3p", bufs=6, space="PSUM") as pp:
        ps_cur = {}
        for bt in plan["batches"]:
            nt = bt["nt"]; t0 = bt["t0"]; chh = bt["chunk"]
            nidx = nt * 128
            T = ep.tile([128, BT * 64], F32, tag="T3")
            Tv = T[:].rearrange("p (k d) -> p k d", d=64)[:, 0:nt, :]
            nc.gpsimd.dma_gather(
                Tv, G[chh * CS:(chh + 1) * CS, :],
                srcsb[:, t0 * 8:(t0 + nt) * 8],
                nidx, nidx, 64, single_packet=False)
            Ter = ep.tile([128, BT * 64], F32, tag="Ter3")
            Terv = Ter[:].rearrange("p (k d) -> p k d", d=64)[:, 0:nt, :]
            nc.gpsimd.dma_gather(
                Terv, Gslab[:, 0:64],
                dstsb[:, t0 * 8:(t0 + nt) * 8],
                nidx, nidx, 64, single_packet=False)
            el = Tv[:, :, 40:41]
            nc.vector.tensor_tensor(out=el, in0=el, in1=Terv[:, :, 41:42],
                                    op=mybir.AluOpType.add)
            lk = ep.tile([128, BT], F32, tag="lk3", name="lk3")
            lkv = lk[:].rearrange("p (k d) -> p k d", d=1)[:, 0:nt, :]
            nc.vector.tensor_scalar_mul(lkv, el, NEG_SLOPE)
            nc.vector.tensor_tensor(out=el, in0=el, in1=lkv,
                                    op=mybir.AluOpType.max)
            nc.scalar.activation(el, el, mybir.ActivationFunctionType.Exp)
            h1 = apx(T[:], 0, [[64, nt], [1, 40]])
            ex1 = apx(T[:], 40, [[64, nt], [0, 40]])
            nc.vector.tensor_tensor(out=h1, in0=h1, in1=ex1,
                                    op=mybir.AluOpType.mult)
            Ssb = sp.tile([128, BT * 128], F32, tag="S3")
            nc.sync.dma_start(Ssb[:, 0:nt * 128], SM[:, t0 * 128:(t0 + nt) * 128])
            for k in range(nt):
                tg = t0 + k
                _, b, tseg = seg_of_tile[tg]
                tin = tile_in_seg[tg]
                if tin == 0:
                    ps_cur[b] = pp.tile([128, 41], F32, tag="ps3", name="ps3")
                ps = ps_cur[b]
                nc.tensor.matmul(
                    ps[:], Ssb[:, k * 128:(k + 1) * 128],
                    T[:, k * 64:k * 64 + 41],
                    start=(tin == 0), stop=(tin == tseg - 1))
                if tin == tseg - 1:
                    nc.vector.tensor_tensor(
                        out=Av[:, b * 41:(b + 1) * 41],
                        in0=Av[:, b * 41:(b + 1) * 41],
                        in1=ps[:], op=mybir.AluOpType.add)
                    del ps_cur[b]
        assert not ps_cur

    with tc.tile_pool(name="n3", bufs=1) as np_:
        O = np_.tile([128, NB * 40], F32, tag="O")
        for b in range(NB):
            Ab = Av[:, b * 41:(b + 1) * 41]
            rs = np_.tile([128, 1], F32, tag="rs3")
            nc.vector.tensor_scalar_max(rs[:], Ab[:, 40:41], 1e-30)
            nc.vector.reciprocal(rs[:], rs[:])
            rsb = apx(rs[:], 0, [[0, 40]])
            Ob = O[:, b * 40:(b + 1) * 40]
            nc.vector.tensor_tensor(out=Ob, in0=Ab[:, 0:40], in1=rsb,
                                    op=mybir.AluOpType.mult)
            nc.vector.tensor_tensor(out=Ob, in0=Ob, in1=b3sb[:],
                                    op=mybir.AluOpType.add)
        Ovv = O[:].rearrange("p (b f) -> p b f", f=40)
        mx = np_.tile([128, NB], F32, tag="mx")
        nc.vector.tensor_reduce(mx[:], Ovv, axis=mybir.AxisListType.X,
                                op=mybir.AluOpType.max)
        mxb = apx(mx[:], 0, [[1, NB], [0, 40]])
        nc.vector.tensor_tensor(out=Ovv, in0=Ovv, in1=mxb,
                                op=mybir.AluOpType.subtract)
        E = np_.tile([128, NB * 40], F32, tag="E")
        nc.scalar.activation(E[:], O[:], mybir.ActivationFunctionType.Exp)
        ss = np_.tile([128, NB], F32, tag="ss")
        nc.vector.tensor_reduce(ss[:], E[:].rearrange("p (b f) -> p b f", f=40),
                                axis=mybir.AxisListType.X, op=mybir.AluOpType.add)
        nc.scalar.activation(ss[:], ss[:], mybir.ActivationFunctionType.Ln)
        ssb = apx(ss[:], 0, [[1, NB], [0, 40]])
        nc.vector.tensor_tensor(out=Ovv, in0=Ovv, in1=ssb,
                                op=mybir.AluOpType.subtract)
        nc.sync.dma_start(OUTT[:, :].rearrange("(b p) f -> p b f", p=128), Ovv)


def make_in_maps(plan, weights, features):
    """Per-core input dicts."""
    C = plan["n_cores"]; NL = plan["NL"]
    features = np.asarray(features, np.float32)
    maps = []
    for c in range(C):
        cd = plan["core_data"][c]
        maps.append(dict(
            featT=np.ascontiguousarray(features[c * NL:(c + 1) * NL].T),
            W1ext=weights["W1ext"], W2ext=weights["W2ext"], W3ext=weights["W3ext"],
            b1rep=weights["b1rep"], b2rep=weights["b2rep"], b3rep=weights["b3rep"],
            src16=cd["src16"], dst16=cd["dst16"], S=cd["S"],
        ))
    return maps


def assemble_output(plan, results):
    C = plan["n_cores"]; NL = plan["NL"]
    outs = [results[c]["out"][:NL] for c in range(C)]
    return np.concatenate(outs, axis=0)


# ---------------- execution harness (PJRT via bass2jax) ----------------
import jax
from jax.sharding import Mesh, PartitionSpec
from jax.experimental.shard_map import shard_map
from concourse.bass2jax import _bass_exec_p, partition_id_tensor, install_neuronx_cc_hook


def build_runner(nc, n_cores):
    install_neuronx_cc_hook()
    partition_name = nc.partition_id_tensor.name if nc.partition_id_tensor else None
    in_names, out_names, out_avals, zero_outs = [], [], [], []
    in_shapes = []
    for alloc in nc.m.functions[0].allocations:
        if not isinstance(alloc, mybir.MemoryLocationSet):
            continue
        name = alloc.memorylocations[0].name
        if alloc.kind == "ExternalInput":
            if name != partition_name and (nc.dbg_addr is None or name != nc.dbg_addr.name):
                in_names.append(name)
                in_shapes.append((tuple(alloc.tensor_shape), mybir.dt.np(alloc.dtype)))
        elif alloc.kind == "ExternalOutput":
            shape = tuple(alloc.tensor_shape)
            dt = mybir.dt.np(alloc.dtype)
            out_names.append(name)
            out_avals.append(jax.core.ShapedArray(shape, dt))
            zero_outs.append(np.zeros(shape, dt))
    n_params = len(in_names)
    n_outs = len(out_names)
    all_in_names = list(in_names) + list(out_names)
    if nc.dbg_addr is not None:
        all_in_names.append(nc.dbg_addr.name)
    if partition_name is not None:
        all_in_names.append(partition_name)

    def _body(*args):
        operands = list(args)
        if nc.dbg_addr is not None:
            operands.append(jax.numpy.zeros((1, 2), jax.numpy.uint32))
        if partition_name is not None:
            operands.append(partition_id_tensor())
        outs = _bass_exec_p.bind(
            *operands,
            out_avals=tuple(out_avals),
            in_names=tuple(all_in_names),
            out_names=tuple(out_names),
            lowering_input_output_aliases=(),
            sim_require_finite=True,
            sim_require_nnan=True,
            nc=nc,
        )
        return tuple(outs)

    devices = jax.devices()[:n_cores]
    mesh = Mesh(np.asarray(devices), ("core",))
    in_specs = (PartitionSpec("core"),) * (n_params + n_outs)
    out_specs = (PartitionSpec("core"),) * n_outs
    sharded = jax.jit(
        shard_map(_body, mesh=mesh, in_specs=in_specs, out_specs=out_specs,
                  check_rep=False),
        keep_unused=True)
    zeros_concat = [np.zeros((n_cores * z.shape[0], *z.shape[1:]), z.dtype)
                    for z in zero_outs]

    from jax.sharding import NamedSharding
    shard = NamedSharding(mesh, PartitionSpec("core"))
    zeros_dev = jax.device_put(zeros_concat, [shard] * len(zeros_concat)) if zeros_concat else []

    in_avals = [jax.ShapeDtypeStruct((n_cores * s[0], *s[1:]), dt, sharding=shard)
                for s, dt in in_shapes]
    out_zero_avals = [jax.ShapeDtypeStruct(z.shape, z.dtype, sharding=shard)
                      for z in zeros_concat]
    compiled = sharded.lower(*in_avals, *out_zero_avals).compile()

    def fn(concat_inputs):
        return compiled(*concat_inputs, *zeros_dev)

    def put(concat_inputs):
        return jax.device_put(concat_inputs, [shard] * len(concat_inputs))

    return fn, in_names, out_names, put, compiled


_CACHE = {}
_LAST = {}


def _get_compiled(plan_key, plan):
    if plan_key not in _CACHE:
        nc = build_program(plan)
        fn, in_names, out_names, put, compiled = build_runner(nc, plan["n_cores"])
        _CACHE[plan_key] = (nc, fn, in_names, out_names, put, compiled)
    return _CACHE[plan_key]


def run_gat(features, weights_kw, src, dst, n_cores=8, n_timing=0):
    n_nodes = features.shape[0]
    plan = host_preprocess(src, dst, n_nodes, n_cores=n_cores, n_chunks=4)
    weights = host_weights(**weights_kw)
    key = (n_nodes, n_cores, bytes(np.asarray(src[:64]).tobytes()),
           plan["n_tiles"])
    nc, fn, in_names, out_names, put, compiled = _get_compiled(key, plan)
    in_maps = make_in_maps(plan, weights, features)
    concat_in = [np.concatenate([np.asarray(in_maps[c][nm])
                                 for c in range(n_cores)], axis=0)
                 for nm in in_names]
    concat_in = put(concat_in)
    _LAST.update(nc=nc, fn=fn, concat_in=concat_in, plan=plan,
                 compiled=compiled, in_names=in_names, out_names=out_names)
    out = fn(concat_in)
    jax.block_until_ready(out)
    times = []
    if n_timing:
        import time
        for _ in range(n_timing):
            t0 = time.perf_counter()
            out = fn(concat_in)
            jax.block_until_ready(out)
            times.append(time.perf_counter() - t0)
    oi = out_names.index("out")
    arr = np.asarray(out[oi])
    NLP = arr.shape[0] // n_cores
    results = [{"out": arr[c * NLP:(c + 1) * NLP]} for c in range(n_cores)]
    full = assemble_output(plan, results)[:n_nodes]
    return full, times


def kernel(features, W1, al1, ar1, b1, W2, al2, ar2, b2, W3, al3, ar3, b3,
           src, dst):
    wk = dict(W1=W1, al1=al1, ar1=ar1, b1=b1, W2=W2, al2=al2, ar2=ar2, b2=b2,
              W3=W3, al3=al3, ar3=ar3, b3=b3)
    out, _ = run_gat(np.asarray(features, np.float32), wk,
                     np.asarray(src), np.asarray(dst), n_cores=8)
    return out.astype(np.float32)



# revision 7
# speedup vs baseline: 2.6360x; 2.6360x over previous
"""GAT Trainium kernel v2: bf16 gathers, on-chip one-hots, matmul-computed er.

v2 changes vs baseline (driven by NTFF profile: GpSimd descriptor generation
for dma_gather was 7.2ms of the 9.16ms exec = ~8ns/index):
- er gathers ELIMINATED (half of all gather indices). Per-edge er is computed
  on the tensor engine: er^T[4k:4k+4, e] = erb_block.T @ S2_tile, where
  S2[d, e] is the (dst x edge) one-hot streamed from host in bf16, and
  erb lives in SBUF [128, NB*heads] (written directly by the node phase - no
  DRAM roundtrip, no dst16 index array at all). One small f32 transpose per
  batch flips er^T back to [e, (t,h)] for the normal-layout score pipeline.
- G rows are 256 bf16 (512B): [h bf16 x128 | el f32 x4 (bitcast) | pad].
  Layer-3 rows are 128 bf16 (256B): [h x40 | el x1 | pad].
- Aggregation one-hot S[e,d] generated ON CHIP per batch:
  S = (dcol[e,t] == iota[d]) on DVE in bf16; host ships dcol [128, NT] bf16
  (pad slots = -1 so they never match).
- All matmuls bf16 (f32 PSUM accumulate); features/weights bf16.
"""
from contextlib import ExitStack
import numpy as np
import ml_dtypes
import concourse.bass as bass
import concourse.tile as tile
from concourse import bacc, mybir
from concourse.masks import make_identity

F32 = mybir.dt.float32
BF16 = mybir.dt.bfloat16
I16 = mybir.dt.int16
BF = ml_dtypes.bfloat16

IN, HID, HEADS, OUT = 256, 32, 4, 40
HH = HID * HEADS  # 128
NEG_SLOPE = 0.2
BT = 16  # tiles per gather batch (16*128 = 2048 idx/call)


def wrap16(a):
    n = a.shape[0]
    assert n % 16 == 0
    blk = a.reshape(-1, 16).T  # [16, n/16]
    return np.tile(blk, (8, 1)).astype(np.int16)


def host_preprocess(src, dst, n_nodes, n_cores=8, n_chunks=4):
    NL = n_nodes // n_cores
    assert NL * n_cores == n_nodes
    NB = (NL + 127) // 128
    CS = (n_nodes + n_chunks - 1) // n_chunks
    assert CS <= 32767

    src = np.asarray(src); dst = np.asarray(dst)
    core_of = dst // NL
    per_core = []
    counts = np.zeros((n_cores, NB, n_chunks), np.int64)
    for c in range(n_cores):
        m = core_of == c
        s, d = src[m], dst[m]
        dloc = d - c * NL
        ch = s // CS
        sloc = s % CS
        order = np.lexsort((dloc, ch))
        dloc, ch, sloc = dloc[order], ch[order], sloc[order]
        b = dloc // 128
        per_core.append((sloc, dloc, ch, b))
        for chh in range(n_chunks):
            mm = ch == chh
            bb, cnt = np.unique(b[mm], return_counts=True)
            counts[c, bb, chh] = cnt
    T = np.ceil(counts.max(axis=0) / 128).astype(np.int64)  # [NB, n_chunks]

    segs = []  # chunk-major: (chunk, block, tiles)
    for chh in range(n_chunks):
        for b in range(NB):
            if T[b, chh] > 0:
                segs.append((chh, b, int(T[b, chh])))
    n_tiles = sum(t for _, _, t in segs)
    total_slots = n_tiles * 128

    batches = []
    cur = None
    tglob = 0
    for chh, b, t in segs:
        for _ in range(t):
            if cur is None or cur["chunk"] != chh or cur["nt"] >= BT:
                if cur is not None:
                    batches.append(cur)
                cur = {"chunk": chh, "t0": tglob, "nt": 0}
            cur["nt"] += 1
            tglob += 1
    if cur is not None:
        batches.append(cur)
    assert tglob == n_tiles

    core_data = []
    for c in range(n_cores):
        sloc, dloc, ch, b = per_core[c]
        src16 = np.zeros(total_slots, np.int16)
        src32 = np.zeros((128, n_tiles), np.int32)  # global src id per slot
        dcol = np.full((128, n_tiles), -1.0, np.float32)  # pad never matches
        S2 = np.zeros((128, total_slots), BF)  # [d, t*128+e] one-hot
        pos = 0
        for chh, bb, t in segs:
            m = (ch == chh) & (b == bb)
            idx = np.nonzero(m)[0]
            n = len(idx)
            cap = t * 128
            assert n <= cap, (c, chh, bb, n, cap)
            sl = sloc[idx]; dl = dloc[idx]
            src16[pos:pos + n] = sl
            e_in_seg = np.arange(n)
            tt = pos // 128 + e_in_seg // 128
            ee = e_in_seg % 128
            src32[ee, tt] = chh * CS + sl
            dloc_in_b = dl - bb * 128
            dcol[ee, tt] = dloc_in_b
            S2[dloc_in_b, tt * 128 + ee] = 1.0
            pos += cap
        assert pos == total_slots
        core_data.append(dict(
            src16=wrap16(src16), src32=src32,
            dcol=dcol.astype(BF),
            S2=S2,
        ))

    return dict(
        n_cores=n_cores, n_nodes=n_nodes, NL=NL, NB=NB, CS=CS,
        n_chunks=n_chunks, segs=segs, batches=batches, n_tiles=n_tiles,
        total_slots=total_slots, core_data=core_data,
    )


def host_weights(W1, al1, ar1, b1, W2, al2, ar2, b2, W3, al3, ar3, b3):
    def bd(al):
        al = np.asarray(al, np.float32)
        H, F = al.shape
        out = np.zeros((H * F, H), np.float32)
        for h in range(H):
            out[h * F:(h + 1) * F, h] = al[h]
        return out
    W1 = np.asarray(W1, np.float32); W2 = np.asarray(W2, np.float32); W3 = np.asarray(W3, np.float32)
    W1ext = np.concatenate([W1, W1 @ bd(al1), W1 @ bd(ar1)], axis=1)
    W2ext = np.concatenate([W2, W2 @ bd(al2), W2 @ bd(ar2)], axis=1)
    W3ext = np.concatenate([W3, W3 @ bd(al3), W3 @ bd(ar3)], axis=1)
    b1rep = np.tile(np.asarray(b1, np.float32).reshape(1, HH), (128, 1))
    b2rep = np.tile(np.asarray(b2, np.float32).reshape(1, HH), (128, 1))
    b3rep = np.tile(np.asarray(b3, np.float32).reshape(1, OUT), (128, 1))
    iota = np.tile(np.arange(128, dtype=np.float32)[None, :], (128, 1))
    return dict(W1ext=W1ext.astype(BF), W2ext=W2ext.astype(BF),
                W3ext=W3ext.astype(BF),
                b1rep=b1rep, b2rep=b2rep, b3rep=b3rep,
                iota=iota.astype(BF))


def apx(base_ap, col_off, dims):
    """AP at column offset of a [128, W] tile with custom free dims."""
    b = base_ap[:, col_off:col_off + 1]
    return bass.AP(b.tensor, b.offset, [b.ap[0]] + [list(d) for d in dims])


def build_program(plan, stage=99):
    C = plan["n_cores"]; NL = plan["NL"]; NB = plan["NB"]
    NT = plan["n_tiles"]; TS = plan["total_slots"]
    NLP = NB * 128

    nc = bacc.Bacc("TRN2", target_bir_lowering=False, debug=False, num_devices=C)

    featT = nc.dram_tensor("featT", [IN, NL], BF16, kind="ExternalInput").ap()
    W1e = nc.dram_tensor("W1ext", [IN, 136], BF16, kind="ExternalInput").ap()
    W2e = nc.dram_tensor("W2ext", [HH, 136], BF16, kind="ExternalInput").ap()
    W3e = nc.dram_tensor("W3ext", [HH, 42], BF16, kind="ExternalInput").ap()
    B1 = nc.dram_tensor("b1rep", [128, HH], F32, kind="ExternalInput").ap()
    B2 = nc.dram_tensor("b2rep", [128, HH], F32, kind="ExternalInput").ap()
    B3 = nc.dram_tensor("b3rep", [128, OUT], F32, kind="ExternalInput").ap()
    SRC = nc.dram_tensor("src16", [128, TS // 16], I16, kind="ExternalInput").ap()
    SRC32 = nc.dram_tensor("src32", [128, NT], mybir.dt.int32, kind="ExternalInput").ap()
    DCOL = nc.dram_tensor("dcol", [128, NT], BF16, kind="ExternalInput").ap()
    IOTA = nc.dram_tensor("iota", [128, 128], BF16, kind="ExternalInput").ap()
    S2D = nc.dram_tensor("S2", [128, TS], BF16, kind="ExternalInput").ap()
    OUTT = nc.dram_tensor("out", [NLP, OUT], F32, kind="ExternalOutput").ap()

    G1s = nc.dram_tensor("G1slab", [NL, 256], BF16).ap()
    G2s = nc.dram_tensor("G2slab", [NL, 256], BF16).ap()
    G3s = nc.dram_tensor("G3slab", [NL, 128], BF16).ap()
    G1 = nc.dram_tensor("G1", [C * NL, 256], BF16, addr_space="Shared").ap()
    G2 = nc.dram_tensor("G2", [C * NL, 256], BF16, addr_space="Shared").ap()
    G3 = nc.dram_tensor("G3", [C * NL, 128], BF16, addr_space="Shared").ap()

    rg = [list(range(C))]

    def allgather(slab, full):
        if C == 1:
            nc.sync.dma_start(full[:, :], slab[:, :])
        else:
            nc.gpsimd.collective_compute(
                "AllGather", mybir.AluOpType.bypass,
                replica_groups=rg, ins=[slab[:, :]], outs=[full[:, :]])

    with tile.TileContext(nc) as tc, ExitStack() as ctx:
        const = ctx.enter_context(tc.tile_pool(name="const", bufs=1))
        accp = ctx.enter_context(tc.tile_pool(name="acc", bufs=1))

        w1sb = const.tile([128, 2 * 136], BF16)
        nc.sync.dma_start(w1sb[:, 0:136], W1e[0:128, :])
        nc.sync.dma_start(w1sb[:, 136:272], W1e[128:256, :])
        w2sb = const.tile([128, 136], BF16)
        nc.sync.dma_start(w2sb[:], W2e[:, :])
        w3sb = const.tile([128, 42], BF16)
        nc.sync.dma_start(w3sb[:], W3e[:, :])
        b1sb = const.tile([128, HH], F32)
        nc.sync.dma_start(b1sb[:], B1[:, :])
        b2sb = const.tile([128, HH], F32)
        nc.sync.dma_start(b2sb[:], B2[:, :])
        b3sb = const.tile([128, OUT], F32)
        nc.sync.dma_start(b3sb[:], B3[:, :])
        ident = const.tile([128, 128], F32)
        make_identity(nc, ident[:])
        iotasb = const.tile([128, 128], BF16)
        nc.sync.dma_start(iotasb[:], IOTA[:, :])
        dcolsb = const.tile([128, NT], BF16)
        nc.sync.dma_start(dcolsb[:], DCOL[:, :])
        srcsb = const.tile([128, TS // 16], I16)
        nc.sync.dma_start(srcsb[:], SRC[:, :])
        src32sb = const.tile([128, NT], mybir.dt.int32)
        nc.sync.dma_start(src32sb[:], SRC32[:, :])
        # SBUF er tables, one per layer: [128 (node-in-block), NB*heads]
        er1sb = const.tile([128, NB * 4], BF16)
        er2sb = const.tile([128, NB * 4], BF16)
        er3sb = const.tile([128, NB], BF16)

        def write_node_rows(lp, ps, nr, b, r0, r1, Gn_s, ersb, n_h, n_el):
            """ps [nr, n_h+n_el+n_er] f32 -> G slab row bf16 + SBUF er table."""
            ncols = Gn_s.shape[1]
            gsb = lp.tile([128, ncols], BF16, tag="gsb")
            nc.vector.tensor_copy(gsb[:nr, 0:n_h], ps[:nr, 0:n_h])
            if n_el == 4:  # f32 el bitcast at bf16 cols [n_h : n_h+8)
                gf = gsb[:].bitcast(F32)
                nc.vector.tensor_copy(gf[:nr, n_h // 2:n_h // 2 + 4],
                                      ps[:nr, n_h:n_h + 4])
                nwr = n_h + 8
            else:  # bf16 el right after h
                nc.vector.tensor_copy(gsb[:nr, n_h:n_h + n_el],
                                      ps[:nr, n_h:n_h + n_el])
                nwr = n_h + n_el
            nc.sync.dma_start(Gn_s[r0:r1, 0:nwr], gsb[:nr, 0:nwr])
            ner = n_el  # heads
            nc.vector.tensor_copy(ersb[:nr, b * ner:(b + 1) * ner],
                                  ps[:nr, n_h + n_el:n_h + n_el + ner])

        # Layer 1 node phase
        with tc.tile_pool(name="l1n", bufs=3) as lp, \
             tc.tile_pool(name="l1np", bufs=2, space="PSUM") as pp:
            for b in range(NB):
                r0 = b * 128
                r1 = min(r0 + 128, NL)
                nr = r1 - r0
                xt = lp.tile([128, 256], BF16, tag="xt")
                nc.sync.dma_start(xt[:, 0:nr], featT[0:128, r0:r1])
                nc.sync.dma_start(xt[:, 128:128 + nr], featT[128:256, r0:r1])
                ps = pp.tile([128, 136], F32, tag="ps")
                nc.tensor.matmul(ps[:nr, :], xt[:, 0:nr], w1sb[:, 0:136],
                                 start=True, stop=False)
                nc.tensor.matmul(ps[:nr, :], xt[:, 128:128 + nr], w1sb[:, 136:272],
                                 start=False, stop=True)
                write_node_rows(lp, ps, nr, b, r0, r1, G1s, er1sb, 128, 4)

        if stage >= 2:
            allgather(G1s, G1)
        if stage >= 3:
            edge_layer(tc, plan, 1, G1, er1sb, (srcsb, src32sb), dcolsb, iotasb, S2D, accp,
                       w_next=w2sb, b_rep=b1sb, Gn_s=G2s, ersb_n=er2sb,
                       ident=ident, write_node=write_node_rows,
                       node_phase=(stage >= 4))
        if stage >= 5:
            allgather(G2s, G2)
            edge_layer(tc, plan, 2, G2, er2sb, (srcsb, src32sb), dcolsb, iotasb, S2D, accp,
                       w_next=w3sb, b_rep=b2sb, Gn_s=G3s, ersb_n=er3sb,
                       ident=ident, write_node=write_node_rows)
            allgather(G3s, G3)
        if stage >= 6:
            edge_layer3(tc, plan, G3, er3sb, (srcsb, src32sb), dcolsb, iotasb, S2D, accp,
                        b3sb, ident, OUTT)

    nc.compile()
    return nc


def seg_maps(plan):
    seg_of_tile = []
    tile_in_seg = []
    for chh, b, t in plan["segs"]:
        seg_of_tile += [(chh, b, t)] * t
        tile_in_seg += list(range(t))
    return seg_of_tile, tile_in_seg


def gen_S(nc, Ssb, dcolsb, iotasb, t0, nt):
    """S[e, k, d] = (dcol[e, t0+k] == iota[d]), bf16, one DVE op."""
    Sv = apx(Ssb[:], 0, [[128, nt], [1, 128]])
    dc = apx(dcolsb[:], t0, [[1, nt], [0, 128]])
    io = apx(iotasb[:], 0, [[0, nt], [1, 128]])
    nc.vector.tensor_tensor(out=Sv, in0=dc, in1=io,
                            op=mybir.AluOpType.is_equal)


def edge_layer(tc, plan, lnum, G, ersb, srcsb, dcolsb, iotasb, S2D, accp,
               w_next, b_rep, Gn_s, ersb_n, ident, write_node,
               node_phase=True):
    nc = tc.nc
    srcsb, src32sb = srcsb
    NL = plan["NL"]; NB = plan["NB"]; CS = plan["CS"]
    seg_of_tile, tile_in_seg = seg_maps(plan)

    A = accp.tile([128, NB * 132], F32, tag="A")
    nc.gpsimd.memset(A[:], 0.0)

    with tc.tile_pool(name=f"e{lnum}", bufs=3) as ep, \
         tc.tile_pool(name=f"e{lnum}b", bufs=3) as bp, \
         tc.tile_pool(name=f"e{lnum}s", bufs=3) as sp, \
         tc.tile_pool(name=f"e{lnum}p", bufs=5, space="PSUM") as pp, \
         tc.tile_pool(name=f"e{lnum}e", bufs=2, space="PSUM") as pe:
        ps_cur = {}
        for bt in plan["batches"]:
            nt = bt["nt"]; t0 = bt["t0"]; chh = bt["chunk"]
            nidx = nt * 128
            T = ep.tile([128, BT * 256], BF16, tag="T")
            Tv = T[:].rearrange("p (k d) -> p k d", d=256)[:, 0:nt, :]
            nc.gpsimd.indirect_dma_start(
                out=Tv, out_offset=None, in_=G[:, :],
                in_offset=bass.IndirectOffsetOnAxis(
                    ap=src32sb[:, t0:t0 + nt], axis=0))
            # S2 one-hot [d, e] stream for er matmuls
            S2sb = sp.tile([128, BT * 128], BF16, tag="S2")
            nc.sync.dma_start(S2sb[:, 0:nt * 128],
                              S2D[:, t0 * 128:(t0 + nt) * 128])
            # er[e, (k,h)] = S2_tile.T @ erb_block   (per tile; lands in the
            # score layout directly - no transpose needed)
            erps = pe.tile([128, BT * 4], F32, tag="erps")
            for k in range(nt):
                _, b, _ = seg_of_tile[t0 + k]
                nc.tensor.matmul(
                    erps[:, k * 4:(k + 1) * 4],
                    S2sb[:, k * 128:(k + 1) * 128],
                    ersb[:, b * 4:(b + 1) * 4],
                    start=True, stop=True)
            # score = el + er ; el is f32 bitcast at f32-cols [64:68) of T rows
            Tf = T[:].bitcast(F32)
            elv = apx(Tf, 64, [[128, nt], [1, 4]])
            erv = erps[:].rearrange("p (k d) -> p k d", d=4)[:, 0:nt, :]
            sc = ep.tile([128, BT * 4], F32, tag="sc", name="sc")
            scv = sc[:].rearrange("p (k d) -> p k d", d=4)[:, 0:nt, :]
            nc.vector.tensor_tensor(out=scv, in0=elv, in1=erv,
                                    op=mybir.AluOpType.add)
            nc.vector.scalar_tensor_tensor(
                out=scv, in0=scv, scalar=NEG_SLOPE, in1=scv,
                op0=mybir.AluOpType.mult, op1=mybir.AluOpType.max)
            B = bp.tile([128, BT * 132], BF16, tag="B")
            Bv = B[:].rearrange("p (k d) -> p k d", d=132)[:, 0:nt, :]
            nc.scalar.activation(Bv[:, :, 128:132], scv,
                                 mybir.ActivationFunctionType.Exp)
            hw = apx(B[:], 0, [[132, nt], [32, 4], [1, 32]])
            hi = apx(T[:], 0, [[256, nt], [32, 4], [1, 32]])
            ex4 = apx(B[:], 128, [[132, nt], [1, 4], [0, 32]])
            nc.vector.tensor_tensor(out=hw, in0=hi, in1=ex4,
                                    op=mybir.AluOpType.mult)
            Ssb = sp.tile([128, BT * 128], BF16, tag="S")
            gen_S(nc, Ssb, dcolsb, iotasb, t0, nt)
            for k in range(nt):
                tg = t0 + k
                _, b, tseg = seg_of_tile[tg]
                tin = tile_in_seg[tg]
                if tin == 0:
                    ps_cur[b] = pp.tile([128, 132], F32, tag="ps", name="ps")
                ps = ps_cur[b]
                nc.tensor.matmul(
                    ps[:], Ssb[:, k * 128:(k + 1) * 128],
                    B[:, k * 132:(k + 1) * 132],
                    start=(tin == 0), stop=(tin == tseg - 1))
                if tin == tseg - 1:
                    nc.vector.tensor_tensor(
                        out=A[:, b * 132:(b + 1) * 132],
                        in0=A[:, b * 132:(b + 1) * 132],
                        in1=ps[:], op=mybir.AluOpType.add)
                    del ps_cur[b]
        assert not ps_cur

    if not node_phase:
        return
    NOUT = w_next.shape[1]
    n_h = {136: 128, 42: 40}[NOUT]
    n_el = {136: 4, 42: 1}[NOUT]
    with tc.tile_pool(name=f"n{lnum}", bufs=3) as np_, \
         tc.tile_pool(name=f"n{lnum}p", bufs=4, space="PSUM") as pp:
        for b in range(NB):
            r0 = b * 128
            r1 = min(r0 + 128, NL)
            nr = r1 - r0
            Ab = A[:, b * 132:(b + 1) * 132]
            rs = np_.tile([128, 4], F32, tag="rs")
            nc.vector.tensor_scalar_max(rs[:], Ab[:, 128:132], 1e-30)
            nc.vector.reciprocal(rs[:], rs[:])
            hp = np_.tile([128, 128], F32, tag="hp")
            hv = hp[:].rearrange("p (g f) -> p g f", g=4)
            rsb = apx(rs[:], 0, [[1, 4], [0, 32]])
            av = Ab[:, 0:128].rearrange("p (g f) -> p g f", g=4)
            nc.vector.tensor_tensor(out=hv, in0=av, in1=rsb,
                                    op=mybir.AluOpType.mult)
            nc.vector.tensor_tensor(out=hp[:], in0=hp[:], in1=b_rep[:],
                                    op=mybir.AluOpType.add)
            nc.scalar.activation(hp[:], hp[:], mybir.ActivationFunctionType.Relu)
            pst = pp.tile([128, 128], F32, tag="pst")
            nc.tensor.transpose(out=pst[:], in_=hp[:], identity=ident[:])
            hpt = np_.tile([128, 128], BF16, tag="hpt")
            nc.vector.tensor_copy(hpt[:], pst[:])
            ps2 = pp.tile([128, NOUT], F32, tag="ps2")
            nc.tensor.matmul(ps2[:nr, :], hpt[:, 0:nr], w_next[:],
                             start=True, stop=True)
            write_node(np_, ps2, nr, b, r0, r1, Gn_s, ersb_n, n_h, n_el)


def edge_layer3(tc, plan, G, ersb, srcsb, dcolsb, iotasb, S2D, accp,
                b3sb, ident, OUTT):
    nc = tc.nc
    srcsb, src32sb = srcsb
    NL = plan["NL"]; NB = plan["NB"]; CS = plan["CS"]
    seg_of_tile, tile_in_seg = seg_maps(plan)

    A = accp.tile([128, NB * 132], F32, tag="A")
    Av = A[:, 0:NB * 41]
    nc.gpsimd.memset(A[:], 0.0)

    with tc.tile_pool(name="e3", bufs=3) as ep, \
         tc.tile_pool(name="e3b", bufs=3) as bp, \
         tc.tile_pool(name="e3s", bufs=3) as sp, \
         tc.tile_pool(name="e3p", bufs=5, space="PSUM") as pp, \
         tc.tile_pool(name="e3e", bufs=2, space="PSUM") as pe:
        ps_cur = {}
        for bt in plan["batches"]:
            nt = bt["nt"]; t0 = bt["t0"]; chh = bt["chunk"]
            nidx = nt * 128
            T = ep.tile([128, BT * 128], BF16, tag="T3")
            Tv = T[:].rearrange("p (k d) -> p k d", d=128)[:, 0:nt, :]
            nc.gpsimd.indirect_dma_start(
                out=Tv, out_offset=None, in_=G[:, :],
                in_offset=bass.IndirectOffsetOnAxis(
                    ap=src32sb[:, t0:t0 + nt], axis=0))
            S2sb = sp.tile([128, BT * 128], BF16, tag="S23")
            nc.sync.dma_start(S2sb[:, 0:nt * 128],
                              S2D[:, t0 * 128:(t0 + nt) * 128])
            erps = pe.tile([128, BT], F32, tag="erps3")
            for k in range(nt):
                _, b, _ = seg_of_tile[t0 + k]
                nc.tensor.matmul(
                    erps[:, k:k + 1],
                    S2sb[:, k * 128:(k + 1) * 128],
                    ersb[:, b:b + 1],
                    start=True, stop=True)
            sc = ep.tile([128, BT], F32, tag="sc3", name="sc3")
            scv = sc[:].rearrange("p (k d) -> p k d", d=1)[:, 0:nt, :]
            erv = erps[:].rearrange("p (k d) -> p k d", d=1)[:, 0:nt, :]
            nc.vector.tensor_tensor(out=scv, in0=Tv[:, :, 40:41], in1=erv,
                                    op=mybir.AluOpType.add)
            nc.vector.scalar_tensor_tensor(
                out=scv, in0=scv, scalar=NEG_SLOPE, in1=scv,
                op0=mybir.AluOpType.mult, op1=mybir.AluOpType.max)
            B = bp.tile([128, BT * 41], BF16, tag="B3")
            Bv = B[:].rearrange("p (k d) -> p k d", d=41)[:, 0:nt, :]
            nc.scalar.activation(Bv[:, :, 40:41], scv,
                                 mybir.ActivationFunctionType.Exp)
            hw = apx(B[:], 0, [[41, nt], [1, 40]])
            hi = apx(T[:], 0, [[128, nt], [1, 40]])
            ex1 = apx(B[:], 40, [[41, nt], [0, 40]])
            nc.vector.tensor_tensor(out=hw, in0=hi, in1=ex1,
                                    op=mybir.AluOpType.mult)
            Ssb = sp.tile([128, BT * 128], BF16, tag="S3")
            gen_S(nc, Ssb, dcolsb, iotasb, t0, nt)
            for k in range(nt):
                tg = t0 + k
                _, b, tseg = seg_of_tile[tg]
                tin = tile_in_seg[tg]
                if tin == 0:
                    ps_cur[b] = pp.tile([128, 41], F32, tag="ps3", name="ps3")
                ps = ps_cur[b]
                nc.tensor.matmul(
                    ps[:], Ssb[:, k * 128:(k + 1) * 128],
                    B[:, k * 41:(k + 1) * 41],
                    start=(tin == 0), stop=(tin == tseg - 1))
                if tin == tseg - 1:
                    nc.vector.tensor_tensor(
                        out=Av[:, b * 41:(b + 1) * 41],
                        in0=Av[:, b * 41:(b + 1) * 41],
                        in1=ps[:], op=mybir.AluOpType.add)
                    del ps_cur[b]
        assert not ps_cur

    with tc.tile_pool(name="n3", bufs=1) as np_:
        O = np_.tile([128, NB * 40], F32, tag="O")
        for b in range(NB):
            Ab = Av[:, b * 41:(b + 1) * 41]
            rs = np_.tile([128, 1], F32, tag="rs3")
            nc.vector.tensor_scalar_max(rs[:], Ab[:, 40:41], 1e-30)
            nc.vector.reciprocal(rs[:], rs[:])
            rsb = apx(rs[:], 0, [[0, 40]])
            Ob = O[:, b * 40:(b + 1) * 40]
            nc.vector.tensor_tensor(out=Ob, in0=Ab[:, 0:40], in1=rsb,
                                    op=mybir.AluOpType.mult)
            nc.vector.tensor_tensor(out=Ob, in0=Ob, in1=b3sb[:],
                                    op=mybir.AluOpType.add)
        Ovv = O[:].rearrange("p (b f) -> p b f", f=40)
        mx = np_.tile([128, NB], F32, tag="mx")
        nc.vector.tensor_reduce(mx[:], Ovv, axis=mybir.AxisListType.X,
                                op=mybir.AluOpType.max)
        mxb = apx(mx[:], 0, [[1, NB], [0, 40]])
        nc.vector.tensor_tensor(out=Ovv, in0=Ovv, in1=mxb,
                                op=mybir.AluOpType.subtract)
        E = np_.tile([128, NB * 40], F32, tag="E")
        nc.scalar.activation(E[:], O[:], mybir.ActivationFunctionType.Exp)
        ss = np_.tile([128, NB], F32, tag="ss")
        nc.vector.tensor_reduce(ss[:], E[:].rearrange("p (b f) -> p b f", f=40),
                                axis=mybir.AxisListType.X, op=mybir.AluOpType.add)
        nc.scalar.activation(ss[:], ss[:], mybir.ActivationFunctionType.Ln)
        ssb = apx(ss[:], 0, [[1, NB], [0, 40]])
        nc.vector.tensor_tensor(out=Ovv, in0=Ovv, in1=ssb,
                                op=mybir.AluOpType.subtract)
        nc.sync.dma_start(OUTT[:, :].rearrange("(b p) f -> p b f", p=128), Ovv)


def make_in_maps(plan, weights, features):
    """Per-core input dicts."""
    C = plan["n_cores"]; NL = plan["NL"]
    features = np.asarray(features, np.float32).astype(BF)
    maps = []
    for c in range(C):
        cd = plan["core_data"][c]
        maps.append(dict(
            featT=np.ascontiguousarray(features[c * NL:(c + 1) * NL].T),
            W1ext=weights["W1ext"], W2ext=weights["W2ext"], W3ext=weights["W3ext"],
            b1rep=weights["b1rep"], b2rep=weights["b2rep"], b3rep=weights["b3rep"],
            iota=weights["iota"],
            src16=cd["src16"], src32=cd["src32"],
            dcol=cd["dcol"], S2=cd["S2"],
        ))
    return maps


def assemble_output(plan, results):
    C = plan["n_cores"]; NL = plan["NL"]
    outs = [results[c]["out"][:NL] for c in range(C)]
    return np.concatenate(outs, axis=0)


# ---------------- execution harness (PJRT via bass2jax) ----------------
import jax
from jax.sharding import Mesh, PartitionSpec
from jax.experimental.shard_map import shard_map
from concourse.bass2jax import _bass_exec_p, partition_id_tensor, install_neuronx_cc_hook


def build_runner(nc, n_cores):
    install_neuronx_cc_hook()
    partition_name = nc.partition_id_tensor.name if nc.partition_id_tensor else None
    in_names, out_names, out_avals, zero_outs = [], [], [], []
    in_shapes = []
    for alloc in nc.m.functions[0].allocations:
        if not isinstance(alloc, mybir.MemoryLocationSet):
            continue
        name = alloc.memorylocations[0].name
        if alloc.kind == "ExternalInput":
            if name != partition_name and (nc.dbg_addr is None or name != nc.dbg_addr.name):
                in_names.append(name)
                in_shapes.append((tuple(alloc.tensor_shape), mybir.dt.np(alloc.dtype)))
        elif alloc.kind == "ExternalOutput":
            shape = tuple(alloc.tensor_shape)
            dt = mybir.dt.np(alloc.dtype)
            out_names.append(name)
            out_avals.append(jax.core.ShapedArray(shape, dt))
            zero_outs.append(np.zeros(shape, dt))
    n_params = len(in_names)
    n_outs = len(out_names)
    all_in_names = list(in_names) + list(out_names)
    if nc.dbg_addr is not None:
        all_in_names.append(nc.dbg_addr.name)
    if partition_name is not None:
        all_in_names.append(partition_name)

    def _body(*args):
        operands = list(args)
        if nc.dbg_addr is not None:
            operands.append(jax.numpy.zeros((1, 2), jax.numpy.uint32))
        if partition_name is not None:
            operands.append(partition_id_tensor())
        outs = _bass_exec_p.bind(
            *operands,
            out_avals=tuple(out_avals),
            in_names=tuple(all_in_names),
            out_names=tuple(out_names),
            lowering_input_output_aliases=(),
            sim_require_finite=True,
            sim_require_nnan=True,
            nc=nc,
        )
        return tuple(outs)

    devices = jax.devices()[:n_cores]
    mesh = Mesh(np.asarray(devices), ("core",))
    in_specs = (PartitionSpec("core"),) * (n_params + n_outs)
    out_specs = (PartitionSpec("core"),) * n_outs
    sharded = jax.jit(
        shard_map(_body, mesh=mesh, in_specs=in_specs, out_specs=out_specs,
                  check_rep=False),
        keep_unused=True)
    zeros_concat = [np.zeros((n_cores * z.shape[0], *z.shape[1:]), z.dtype)
                    for z in zero_outs]

    from jax.sharding import NamedSharding
    shard = NamedSharding(mesh, PartitionSpec("core"))
    zeros_dev = jax.device_put(zeros_concat, [shard] * len(zeros_concat)) if zeros_concat else []

    in_avals = [jax.ShapeDtypeStruct((n_cores * s[0], *s[1:]), dt, sharding=shard)
                for s, dt in in_shapes]
    out_zero_avals = [jax.ShapeDtypeStruct(z.shape, z.dtype, sharding=shard)
                      for z in zeros_concat]
    compiled = sharded.lower(*in_avals, *out_zero_avals).compile()

    def fn(concat_inputs):
        return compiled(*concat_inputs, *zeros_dev)

    def put(concat_inputs):
        return jax.device_put(concat_inputs, [shard] * len(concat_inputs))

    return fn, in_names, out_names, put, compiled


_CACHE = {}
_LAST = {}


def _get_compiled(plan_key, plan):
    if plan_key not in _CACHE:
        nc = build_program(plan)
        fn, in_names, out_names, put, compiled = build_runner(nc, plan["n_cores"])
        _CACHE[plan_key] = (nc, fn, in_names, out_names, put, compiled)
    return _CACHE[plan_key]


def run_gat(features, weights_kw, src, dst, n_cores=8, n_timing=0):
    n_nodes = features.shape[0]
    plan = host_preprocess(src, dst, n_nodes, n_cores=n_cores, n_chunks=4)
    weights = host_weights(**weights_kw)
    key = (n_nodes, n_cores, bytes(np.asarray(src[:64]).tobytes()),
           plan["n_tiles"])
    nc, fn, in_names, out_names, put, compiled = _get_compiled(key, plan)
    in_maps = make_in_maps(plan, weights, features)
    concat_in = [np.concatenate([np.asarray(in_maps[c][nm])
                                 for c in range(n_cores)], axis=0)
                 for nm in in_names]
    concat_in = put(concat_in)
    _LAST.update(nc=nc, fn=fn, concat_in=concat_in, plan=plan,
                 compiled=compiled, in_names=in_names, out_names=out_names)
    out = fn(concat_in)
    jax.block_until_ready(out)
    times = []
    if n_timing:
        import time
        for _ in range(n_timing):
            t0 = time.perf_counter()
            out = fn(concat_in)
            jax.block_until_ready(out)
            times.append(time.perf_counter() - t0)
    oi = out_names.index("out")
    arr = np.asarray(out[oi])
    NLP = arr.shape[0] // n_cores
    results = [{"out": arr[c * NLP:(c + 1) * NLP]} for c in range(n_cores)]
    full = assemble_output(plan, results)[:n_nodes]
    return full, times


def kernel(features, W1, al1, ar1, b1, W2, al2, ar2, b2, W3, al3, ar3, b3,
           src, dst):
    wk = dict(W1=W1, al1=al1, ar1=ar1, b1=b1, W2=W2, al2=al2, ar2=ar2, b2=b2,
              W3=W3, al3=al3, ar3=ar3, b3=b3)
    out, _ = run_gat(np.asarray(features, np.float32), wk,
                     np.asarray(src), np.asarray(dst), n_cores=8)
    return out.astype(np.float32)


# revision 9
# speedup vs baseline: 10.9007x; 4.1353x over previous
"""GAT Trainium kernel v2: bf16 gathers, on-chip one-hots, matmul-computed er.

v2 changes vs baseline (driven by NTFF profile: GpSimd descriptor generation
for dma_gather was 7.2ms of the 9.16ms exec = ~8ns/index):
- er gathers ELIMINATED (half of all gather indices). Per-edge er is computed
  on the tensor engine: er^T[4k:4k+4, e] = erb_block.T @ S2_tile, where
  S2[d, e] is the (dst x edge) one-hot streamed from host in bf16, and
  erb lives in SBUF [128, NB*heads] (written directly by the node phase - no
  DRAM roundtrip, no dst16 index array at all). One small f32 transpose per
  batch flips er^T back to [e, (t,h)] for the normal-layout score pipeline.
- G rows are 256 bf16 (512B): [h bf16 x128 | el f32 x4 (bitcast) | pad].
  Layer-3 rows are 128 bf16 (256B): [h x40 | el x1 | pad].
- Aggregation one-hot S[e,d] generated ON CHIP per batch:
  S = (dcol[e,t] == iota[d]) on DVE in bf16; host ships dcol [128, NT] bf16
  (pad slots = -1 so they never match).
- All matmuls bf16 (f32 PSUM accumulate); features/weights bf16.
"""
from contextlib import ExitStack
import numpy as np
import ml_dtypes
import concourse.bass as bass
import concourse.tile as tile
from concourse import bacc, mybir
from concourse.masks import make_identity

F32 = mybir.dt.float32
BF16 = mybir.dt.bfloat16
I16 = mybir.dt.int16
BF = ml_dtypes.bfloat16

IN, HID, HEADS, OUT = 256, 32, 4, 40
HH = HID * HEADS  # 128
NEG_SLOPE = 0.2
BT = 16  # tiles per gather batch (16*128 = 2048 idx/call)


def wrap16(a):
    n = a.shape[0]
    assert n % 16 == 0
    blk = a.reshape(-1, 16).T  # [16, n/16]
    return np.tile(blk, (8, 1)).astype(np.int16)


def host_preprocess(src, dst, n_nodes, n_cores=8, n_chunks=4):
    NL = n_nodes // n_cores
    assert NL * n_cores == n_nodes
    NB = (NL + 127) // 128
    CS = (n_nodes + n_chunks - 1) // n_chunks
    assert CS <= 32767

    src = np.asarray(src); dst = np.asarray(dst)
    core_of = dst // NL
    per_core = []
    counts = np.zeros((n_cores, NB, n_chunks), np.int64)
    for c in range(n_cores):
        m = core_of == c
        s, d = src[m], dst[m]
        dloc = d - c * NL
        ch = s // CS
        sloc = s % CS
        order = np.lexsort((dloc, ch))
        dloc, ch, sloc = dloc[order], ch[order], sloc[order]
        b = dloc // 128
        per_core.append((sloc, dloc, ch, b))
        for chh in range(n_chunks):
            mm = ch == chh
            bb, cnt = np.unique(b[mm], return_counts=True)
            counts[c, bb, chh] = cnt
    T = np.ceil(counts.max(axis=0) / 128).astype(np.int64)  # [NB, n_chunks]

    segs = []  # chunk-major: (chunk, block, tiles)
    for chh in range(n_chunks):
        for b in range(NB):
            if T[b, chh] > 0:
                segs.append((chh, b, int(T[b, chh])))
    n_tiles = sum(t for _, _, t in segs)
    total_slots = n_tiles * 128

    batches = []
    cur = None
    tglob = 0
    for chh, b, t in segs:
        for _ in range(t):
            if cur is None or cur["chunk"] != chh or cur["nt"] >= BT:
                if cur is not None:
                    batches.append(cur)
                cur = {"chunk": chh, "t0": tglob, "nt": 0}
            cur["nt"] += 1
            tglob += 1
    if cur is not None:
        batches.append(cur)
    assert tglob == n_tiles

    core_data = []
    for c in range(n_cores):
        sloc, dloc, ch, b = per_core[c]
        src16 = np.zeros(total_slots, np.int16)
        src32 = np.zeros((128, n_tiles), np.int32)  # global src id per slot
        dcol = np.full((128, n_tiles), -1.0, np.float32)  # pad never matches
        S2 = np.zeros((128, total_slots), BF)  # [d, t*128+e] one-hot
        pos = 0
        for chh, bb, t in segs:
            m = (ch == chh) & (b == bb)
            idx = np.nonzero(m)[0]
            n = len(idx)
            cap = t * 128
            assert n <= cap, (c, chh, bb, n, cap)
            sl = sloc[idx]; dl = dloc[idx]
            src16[pos:pos + n] = sl
            e_in_seg = np.arange(n)
            tt = pos // 128 + e_in_seg // 128
            ee = e_in_seg % 128
            src32[ee, tt] = chh * CS + sl
            dloc_in_b = dl - bb * 128
            dcol[ee, tt] = dloc_in_b
            S2[dloc_in_b, tt * 128 + ee] = 1.0
            pos += cap
        assert pos == total_slots
        core_data.append(dict(
            src16=wrap16(src16), src32=src32,
            dcol=dcol.astype(BF),
            S2=S2,
        ))

    return dict(
        n_cores=n_cores, n_nodes=n_nodes, NL=NL, NB=NB, CS=CS,
        n_chunks=n_chunks, segs=segs, batches=batches, n_tiles=n_tiles,
        total_slots=total_slots, core_data=core_data,
    )


def host_weights(W1, al1, ar1, b1, W2, al2, ar2, b2, W3, al3, ar3, b3):
    def bd(al):
        al = np.asarray(al, np.float32)
        H, F = al.shape
        out = np.zeros((H * F, H), np.float32)
        for h in range(H):
            out[h * F:(h + 1) * F, h] = al[h]
        return out
    W1 = np.asarray(W1, np.float32); W2 = np.asarray(W2, np.float32); W3 = np.asarray(W3, np.float32)
    W1ext = np.concatenate([W1, W1 @ bd(al1), W1 @ bd(ar1)], axis=1)
    W2ext = np.concatenate([W2, W2 @ bd(al2), W2 @ bd(ar2)], axis=1)
    W3ext = np.concatenate([W3, W3 @ bd(al3), W3 @ bd(ar3)], axis=1)
    b1rep = np.tile(np.asarray(b1, np.float32).reshape(1, HH), (128, 1))
    b2rep = np.tile(np.asarray(b2, np.float32).reshape(1, HH), (128, 1))
    b3rep = np.tile(np.asarray(b3, np.float32).reshape(1, OUT), (128, 1))
    iota = np.tile(np.arange(128, dtype=np.float32)[None, :], (128, 1))
    return dict(W1ext=W1ext.astype(BF), W2ext=W2ext.astype(BF),
                W3ext=W3ext.astype(BF),
                b1rep=b1rep, b2rep=b2rep, b3rep=b3rep,
                iota=iota.astype(BF))


def apx(base_ap, col_off, dims):
    """AP at column offset of a [128, W] tile with custom free dims."""
    b = base_ap[:, col_off:col_off + 1]
    return bass.AP(b.tensor, b.offset, [b.ap[0]] + [list(d) for d in dims])


def build_program(plan, stage=99):
    C = plan["n_cores"]; NL = plan["NL"]; NB = plan["NB"]
    NT = plan["n_tiles"]; TS = plan["total_slots"]
    NLP = NB * 128

    nc = bacc.Bacc("TRN2", target_bir_lowering=False, debug=False, num_devices=C)

    featT = nc.dram_tensor("featT", [IN, NL], BF16, kind="ExternalInput").ap()
    W1e = nc.dram_tensor("W1ext", [IN, 136], BF16, kind="ExternalInput").ap()
    W2e = nc.dram_tensor("W2ext", [HH, 136], BF16, kind="ExternalInput").ap()
    W3e = nc.dram_tensor("W3ext", [HH, 42], BF16, kind="ExternalInput").ap()
    B1 = nc.dram_tensor("b1rep", [128, HH], F32, kind="ExternalInput").ap()
    B2 = nc.dram_tensor("b2rep", [128, HH], F32, kind="ExternalInput").ap()
    B3 = nc.dram_tensor("b3rep", [128, OUT], F32, kind="ExternalInput").ap()
    SRC = nc.dram_tensor("src16", [128, TS // 16], I16, kind="ExternalInput").ap()
    SRC32 = nc.dram_tensor("src32", [128, NT], mybir.dt.int32, kind="ExternalInput").ap()
    DCOL = nc.dram_tensor("dcol", [128, NT], BF16, kind="ExternalInput").ap()
    IOTA = nc.dram_tensor("iota", [128, 128], BF16, kind="ExternalInput").ap()
    S2D = nc.dram_tensor("S2", [128, TS], BF16, kind="ExternalInput").ap()
    OUTT = nc.dram_tensor("out", [NLP, OUT], F32, kind="ExternalOutput").ap()

    G1s = nc.dram_tensor("G1slab", [NL, 256], BF16).ap()
    G2s = nc.dram_tensor("G2slab", [NL, 256], BF16).ap()
    G3s = nc.dram_tensor("G3slab", [NL, 128], BF16).ap()
    G1 = nc.dram_tensor("G1", [C * NL, 256], BF16, addr_space="Shared").ap()
    G2 = nc.dram_tensor("G2", [C * NL, 256], BF16, addr_space="Shared").ap()
    G3 = nc.dram_tensor("G3", [C * NL, 128], BF16, addr_space="Shared").ap()

    rg = [list(range(C))]

    def allgather(slab, full):
        if C == 1:
            nc.sync.dma_start(full[:, :], slab[:, :])
        else:
            nc.gpsimd.collective_compute(
                "AllGather", mybir.AluOpType.bypass,
                replica_groups=rg, ins=[slab[:, :]], outs=[full[:, :]])

    with tile.TileContext(nc) as tc, ExitStack() as ctx:
        const = ctx.enter_context(tc.tile_pool(name="const", bufs=1))
        accp = ctx.enter_context(tc.tile_pool(name="acc", bufs=1))

        w1sb = const.tile([128, 2 * 136], BF16)
        nc.sync.dma_start(w1sb[:, 0:136], W1e[0:128, :])
        nc.sync.dma_start(w1sb[:, 136:272], W1e[128:256, :])
        w2sb = const.tile([128, 136], BF16)
        nc.sync.dma_start(w2sb[:], W2e[:, :])
        w3sb = const.tile([128, 42], BF16)
        nc.sync.dma_start(w3sb[:], W3e[:, :])
        b1sb = const.tile([128, HH], F32)
        nc.sync.dma_start(b1sb[:], B1[:, :])
        b2sb = const.tile([128, HH], F32)
        nc.sync.dma_start(b2sb[:], B2[:, :])
        b3sb = const.tile([128, OUT], F32)
        nc.sync.dma_start(b3sb[:], B3[:, :])
        ident = const.tile([128, 128], F32)
        make_identity(nc, ident[:])
        iotasb = const.tile([128, 128], BF16)
        nc.sync.dma_start(iotasb[:], IOTA[:, :])
        dcolsb = const.tile([128, NT], BF16)
        nc.sync.dma_start(dcolsb[:], DCOL[:, :])
        srcsb = const.tile([128, TS // 16], I16)
        nc.sync.dma_start(srcsb[:], SRC[:, :])
        src32sb = const.tile([128, NT], mybir.dt.int32)
        nc.sync.dma_start(src32sb[:], SRC32[:, :])
        # SBUF er tables, one per layer: [128 (node-in-block), NB*heads]
        er1sb = const.tile([128, NB * 4], BF16)
        er2sb = const.tile([128, NB * 4], BF16)
        er3sb = const.tile([128, NB], BF16)

        def write_node_rows(lp, ps, nr, b, r0, r1, Gn_s, ersb, n_h, n_el):
            """ps [nr, n_h+n_el+n_er] f32 -> G slab row bf16 + SBUF er table."""
            ncols = Gn_s.shape[1]
            gsb = lp.tile([128, ncols], BF16, tag="gsb")
            nc.vector.tensor_copy(gsb[:nr, 0:n_h], ps[:nr, 0:n_h])
            if n_el == 4:  # f32 el bitcast at bf16 cols [n_h : n_h+8)
                gf = gsb[:].bitcast(F32)
                nc.vector.tensor_copy(gf[:nr, n_h // 2:n_h // 2 + 4],
                                      ps[:nr, n_h:n_h + 4])
                nwr = n_h + 8
            else:  # bf16 el right after h
                nc.vector.tensor_copy(gsb[:nr, n_h:n_h + n_el],
                                      ps[:nr, n_h:n_h + n_el])
                nwr = n_h + n_el
            nc.sync.dma_start(Gn_s[r0:r1, 0:nwr], gsb[:nr, 0:nwr])
            ner = n_el  # heads
            nc.vector.tensor_copy(ersb[:nr, b * ner:(b + 1) * ner],
                                  ps[:nr, n_h + n_el:n_h + n_el + ner])

        # Layer 1 node phase
        with tc.tile_pool(name="l1n", bufs=3) as lp, \
             tc.tile_pool(name="l1np", bufs=2, space="PSUM") as pp:
            for b in range(NB):
                r0 = b * 128
                r1 = min(r0 + 128, NL)
                nr = r1 - r0
                xt = lp.tile([128, 256], BF16, tag="xt")
                nc.sync.dma_start(xt[:, 0:nr], featT[0:128, r0:r1])
                nc.sync.dma_start(xt[:, 128:128 + nr], featT[128:256, r0:r1])
                ps = pp.tile([128, 136], F32, tag="ps")
                nc.tensor.matmul(ps[:nr, :], xt[:, 0:nr], w1sb[:, 0:136],
                                 start=True, stop=False)
                nc.tensor.matmul(ps[:nr, :], xt[:, 128:128 + nr], w1sb[:, 136:272],
                                 start=False, stop=True)
                write_node_rows(lp, ps, nr, b, r0, r1, G1s, er1sb, 128, 4)

        if stage >= 2:
            allgather(G1s, G1)
        if stage >= 3:
            edge_layer(tc, plan, 1, G1, er1sb, (srcsb, src32sb), dcolsb, iotasb, S2D, accp,
                       w_next=w2sb, b_rep=b1sb, Gn_s=G2s, ersb_n=er2sb,
                       ident=ident, write_node=write_node_rows,
                       node_phase=(stage >= 4))
        if stage >= 5:
            allgather(G2s, G2)
            edge_layer(tc, plan, 2, G2, er2sb, (srcsb, src32sb), dcolsb, iotasb, S2D, accp,
                       w_next=w3sb, b_rep=b2sb, Gn_s=G3s, ersb_n=er3sb,
                       ident=ident, write_node=write_node_rows)
            allgather(G3s, G3)
        if stage >= 6:
            edge_layer3(tc, plan, G3, er3sb, (srcsb, src32sb), dcolsb, iotasb, S2D, accp,
                        b3sb, ident, OUTT)

    nc.compile()
    return nc


def seg_maps(plan):
    seg_of_tile = []
    tile_in_seg = []
    for chh, b, t in plan["segs"]:
        seg_of_tile += [(chh, b, t)] * t
        tile_in_seg += list(range(t))
    return seg_of_tile, tile_in_seg


def gen_S(nc, Ssb, dcolsb, iotasb, t0, nt):
    """S[e, k, d] = (dcol[e, t0+k] == iota[d]), bf16, one DVE op."""
    Sv = apx(Ssb[:], 0, [[128, nt], [1, 128]])
    dc = apx(dcolsb[:], t0, [[1, nt], [0, 128]])
    io = apx(iotasb[:], 0, [[0, nt], [1, 128]])
    nc.vector.tensor_tensor(out=Sv, in0=dc, in1=io,
                            op=mybir.AluOpType.is_equal)


def edge_layer(tc, plan, lnum, G, ersb, srcsb, dcolsb, iotasb, S2D, accp,
               w_next, b_rep, Gn_s, ersb_n, ident, write_node,
               node_phase=True):
    nc = tc.nc
    srcsb, src32sb = srcsb
    NL = plan["NL"]; NB = plan["NB"]; CS = plan["CS"]
    seg_of_tile, tile_in_seg = seg_maps(plan)

    A = accp.tile([128, NB * 132], F32, tag="A")
    nc.gpsimd.memset(A[:], 0.0)

    with tc.tile_pool(name=f"e{lnum}", bufs=5) as ep, \
         tc.tile_pool(name=f"e{lnum}b", bufs=4) as bp, \
         tc.tile_pool(name=f"e{lnum}s", bufs=4) as sp, \
         tc.tile_pool(name=f"e{lnum}p", bufs=5, space="PSUM") as pp, \
         tc.tile_pool(name=f"e{lnum}e", bufs=2, space="PSUM") as pe:
        ps_cur = {}
        for bt in plan["batches"]:
            nt = bt["nt"]; t0 = bt["t0"]; chh = bt["chunk"]
            nidx = nt * 128
            T = ep.tile([128, BT * 256], BF16, tag="T")
            Tv = T[:].rearrange("p (k d) -> p k d", d=256)[:, 0:nt, :]
            nc.gpsimd.dma_gather(
                Tv, G[chh * CS:(chh + 1) * CS, :],
                srcsb[:, t0 * 8:(t0 + nt) * 8],
                nidx, nidx, 256, single_packet=False)
            # S2 one-hot [d, e] stream for er matmuls
            S2sb = sp.tile([128, BT * 128], BF16, tag="S2")
            nc.sync.dma_start(S2sb[:, 0:nt * 128],
                              S2D[:, t0 * 128:(t0 + nt) * 128])
            # er[e, (k,h)] = S2_tile.T @ erb_block   (per tile; lands in the
            # score layout directly - no transpose needed)
            erps = pe.tile([128, BT * 4], F32, tag="erps")
            for k in range(nt):
                _, b, _ = seg_of_tile[t0 + k]
                nc.tensor.matmul(
                    erps[:, k * 4:(k + 1) * 4],
                    S2sb[:, k * 128:(k + 1) * 128],
                    ersb[:, b * 4:(b + 1) * 4],
                    start=True, stop=True)
            # score = el + er ; el is f32 bitcast at f32-cols [64:68) of T rows
            Tf = T[:].bitcast(F32)
            elv = apx(Tf, 64, [[128, nt], [1, 4]])
            erv = erps[:].rearrange("p (k d) -> p k d", d=4)[:, 0:nt, :]
            sc = ep.tile([128, BT * 4], F32, tag="sc", name="sc")
            scv = sc[:].rearrange("p (k d) -> p k d", d=4)[:, 0:nt, :]
            nc.vector.tensor_tensor(out=scv, in0=elv, in1=erv,
                                    op=mybir.AluOpType.add)
            nc.vector.scalar_tensor_tensor(
                out=scv, in0=scv, scalar=NEG_SLOPE, in1=scv,
                op0=mybir.AluOpType.mult, op1=mybir.AluOpType.max)
            B = bp.tile([128, BT * 132], BF16, tag="B")
            Bv = B[:].rearrange("p (k d) -> p k d", d=132)[:, 0:nt, :]
            nc.scalar.activation(Bv[:, :, 128:132], scv,
                                 mybir.ActivationFunctionType.Exp)
            hw = apx(B[:], 0, [[132, nt], [32, 4], [1, 32]])
            hi = apx(T[:], 0, [[256, nt], [32, 4], [1, 32]])
            ex4 = apx(B[:], 128, [[132, nt], [1, 4], [0, 32]])
            nc.vector.tensor_tensor(out=hw, in0=hi, in1=ex4,
                                    op=mybir.AluOpType.mult)
            Ssb = sp.tile([128, BT * 128], BF16, tag="S")
            gen_S(nc, Ssb, dcolsb, iotasb, t0, nt)
            for k in range(nt):
                tg = t0 + k
                _, b, tseg = seg_of_tile[tg]
                tin = tile_in_seg[tg]
                if tin == 0:
                    ps_cur[b] = pp.tile([128, 132], F32, tag="ps", name="ps")
                ps = ps_cur[b]
                nc.tensor.matmul(
                    ps[:], Ssb[:, k * 128:(k + 1) * 128],
                    B[:, k * 132:(k + 1) * 132],
                    start=(tin == 0), stop=(tin == tseg - 1))
                if tin == tseg - 1:
                    nc.vector.tensor_tensor(
                        out=A[:, b * 132:(b + 1) * 132],
                        in0=A[:, b * 132:(b + 1) * 132],
                        in1=ps[:], op=mybir.AluOpType.add)
                    del ps_cur[b]
        assert not ps_cur

    if not node_phase:
        return
    NOUT = w_next.shape[1]
    n_h = {136: 128, 42: 40}[NOUT]
    n_el = {136: 4, 42: 1}[NOUT]
    with tc.tile_pool(name=f"n{lnum}", bufs=3) as np_, \
         tc.tile_pool(name=f"n{lnum}p", bufs=4, space="PSUM") as pp:
        for b in range(NB):
            r0 = b * 128
            r1 = min(r0 + 128, NL)
            nr = r1 - r0
            Ab = A[:, b * 132:(b + 1) * 132]
            rs = np_.tile([128, 4], F32, tag="rs")
            nc.vector.tensor_scalar_max(rs[:], Ab[:, 128:132], 1e-30)
            nc.vector.reciprocal(rs[:], rs[:])
            hp = np_.tile([128, 128], F32, tag="hp")
            hv = hp[:].rearrange("p (g f) -> p g f", g=4)
            rsb = apx(rs[:], 0, [[1, 4], [0, 32]])
            av = Ab[:, 0:128].rearrange("p (g f) -> p g f", g=4)
            nc.vector.tensor_tensor(out=hv, in0=av, in1=rsb,
                                    op=mybir.AluOpType.mult)
            nc.vector.tensor_tensor(out=hp[:], in0=hp[:], in1=b_rep[:],
                                    op=mybir.AluOpType.add)
            nc.scalar.activation(hp[:], hp[:], mybir.ActivationFunctionType.Relu)
            pst = pp.tile([128, 128], F32, tag="pst")
            nc.tensor.transpose(out=pst[:], in_=hp[:], identity=ident[:])
            hpt = np_.tile([128, 128], BF16, tag="hpt")
            nc.vector.tensor_copy(hpt[:], pst[:])
            ps2 = pp.tile([128, NOUT], F32, tag="ps2")
            nc.tensor.matmul(ps2[:nr, :], hpt[:, 0:nr], w_next[:],
                             start=True, stop=True)
            write_node(np_, ps2, nr, b, r0, r1, Gn_s, ersb_n, n_h, n_el)


def edge_layer3(tc, plan, G, ersb, srcsb, dcolsb, iotasb, S2D, accp,
                b3sb, ident, OUTT):
    nc = tc.nc
    srcsb, src32sb = srcsb
    NL = plan["NL"]; NB = plan["NB"]; CS = plan["CS"]
    seg_of_tile, tile_in_seg = seg_maps(plan)

    A = accp.tile([128, NB * 132], F32, tag="A")
    Av = A[:, 0:NB * 41]
    nc.gpsimd.memset(A[:], 0.0)

    with tc.tile_pool(name="e3", bufs=5) as ep, \
         tc.tile_pool(name="e3b", bufs=4) as bp, \
         tc.tile_pool(name="e3s", bufs=4) as sp, \
         tc.tile_pool(name="e3p", bufs=5, space="PSUM") as pp, \
         tc.tile_pool(name="e3e", bufs=2, space="PSUM") as pe:
        ps_cur = {}
        for bt in plan["batches"]:
            nt = bt["nt"]; t0 = bt["t0"]; chh = bt["chunk"]
            nidx = nt * 128
            T = ep.tile([128, BT * 128], BF16, tag="T3")
            Tv = T[:].rearrange("p (k d) -> p k d", d=128)[:, 0:nt, :]
            nc.gpsimd.dma_gather(
                Tv, G[chh * CS:(chh + 1) * CS, :],
                srcsb[:, t0 * 8:(t0 + nt) * 8],
                nidx, nidx, 128, single_packet=False)
            S2sb = sp.tile([128, BT * 128], BF16, tag="S23")
            nc.sync.dma_start(S2sb[:, 0:nt * 128],
                              S2D[:, t0 * 128:(t0 + nt) * 128])
            erps = pe.tile([128, BT], F32, tag="erps3")
            for k in range(nt):
                _, b, _ = seg_of_tile[t0 + k]
                nc.tensor.matmul(
                    erps[:, k:k + 1],
                    S2sb[:, k * 128:(k + 1) * 128],
                    ersb[:, b:b + 1],
                    start=True, stop=True)
            sc = ep.tile([128, BT], F32, tag="sc3", name="sc3")
            scv = sc[:].rearrange("p (k d) -> p k d", d=1)[:, 0:nt, :]
            erv = erps[:].rearrange("p (k d) -> p k d", d=1)[:, 0:nt, :]
            nc.vector.tensor_tensor(out=scv, in0=Tv[:, :, 40:41], in1=erv,
                                    op=mybir.AluOpType.add)
            nc.vector.scalar_tensor_tensor(
                out=scv, in0=scv, scalar=NEG_SLOPE, in1=scv,
                op0=mybir.AluOpType.mult, op1=mybir.AluOpType.max)
            B = bp.tile([128, BT * 41], BF16, tag="B3")
            Bv = B[:].rearrange("p (k d) -> p k d", d=41)[:, 0:nt, :]
            nc.scalar.activation(Bv[:, :, 40:41], scv,
                                 mybir.ActivationFunctionType.Exp)
            hw = apx(B[:], 0, [[41, nt], [1, 40]])
            hi = apx(T[:], 0, [[128, nt], [1, 40]])
            ex1 = apx(B[:], 40, [[41, nt], [0, 40]])
            nc.vector.tensor_tensor(out=hw, in0=hi, in1=ex1,
                                    op=mybir.AluOpType.mult)
            Ssb = sp.tile([128, BT * 128], BF16, tag="S3")
            gen_S(nc, Ssb, dcolsb, iotasb, t0, nt)
            for k in range(nt):
                tg = t0 + k
                _, b, tseg = seg_of_tile[tg]
                tin = tile_in_seg[tg]
                if tin == 0:
                    ps_cur[b] = pp.tile([128, 41], F32, tag="ps3", name="ps3")
                ps = ps_cur[b]
                nc.tensor.matmul(
                    ps[:], Ssb[:, k * 128:(k + 1) * 128],
                    B[:, k * 41:(k + 1) * 41],
                    start=(tin == 0), stop=(tin == tseg - 1))
                if tin == tseg - 1:
                    nc.vector.tensor_tensor(
                        out=Av[:, b * 41:(b + 1) * 41],
                        in0=Av[:, b * 41:(b + 1) * 41],
                        in1=ps[:], op=mybir.AluOpType.add)
                    del ps_cur[b]
        assert not ps_cur

    with tc.tile_pool(name="n3", bufs=1) as np_:
        O = np_.tile([128, NB * 40], F32, tag="O")
        for b in range(NB):
            Ab = Av[:, b * 41:(b + 1) * 41]
            rs = np_.tile([128, 1], F32, tag="rs3")
            nc.vector.tensor_scalar_max(rs[:], Ab[:, 40:41], 1e-30)
            nc.vector.reciprocal(rs[:], rs[:])
            rsb = apx(rs[:], 0, [[0, 40]])
            Ob = O[:, b * 40:(b + 1) * 40]
            nc.vector.tensor_tensor(out=Ob, in0=Ab[:, 0:40], in1=rsb,
                                    op=mybir.AluOpType.mult)
            nc.vector.tensor_tensor(out=Ob, in0=Ob, in1=b3sb[:],
                                    op=mybir.AluOpType.add)
        Ovv = O[:].rearrange("p (b f) -> p b f", f=40)
        mx = np_.tile([128, NB], F32, tag="mx")
        nc.vector.tensor_reduce(mx[:], Ovv, axis=mybir.AxisListType.X,
                                op=mybir.AluOpType.max)
        mxb = apx(mx[:], 0, [[1, NB], [0, 40]])
        nc.vector.tensor_tensor(out=Ovv, in0=Ovv, in1=mxb,
                                op=mybir.AluOpType.subtract)
        E = np_.tile([128, NB * 40], F32, tag="E")
        nc.scalar.activation(E[:], O[:], mybir.ActivationFunctionType.Exp)
        ss = np_.tile([128, NB], F32, tag="ss")
        nc.vector.tensor_reduce(ss[:], E[:].rearrange("p (b f) -> p b f", f=40),
                                axis=mybir.AxisListType.X, op=mybir.AluOpType.add)
        nc.scalar.activation(ss[:], ss[:], mybir.ActivationFunctionType.Ln)
        ssb = apx(ss[:], 0, [[1, NB], [0, 40]])
        nc.vector.tensor_tensor(out=Ovv, in0=Ovv, in1=ssb,
                                op=mybir.AluOpType.subtract)
        nc.sync.dma_start(OUTT[:, :].rearrange("(b p) f -> p b f", p=128), Ovv)


def make_in_maps(plan, weights, features):
    """Per-core input dicts."""
    C = plan["n_cores"]; NL = plan["NL"]
    features = np.asarray(features, np.float32).astype(BF)
    maps = []
    for c in range(C):
        cd = plan["core_data"][c]
        maps.append(dict(
            featT=np.ascontiguousarray(features[c * NL:(c + 1) * NL].T),
            W1ext=weights["W1ext"], W2ext=weights["W2ext"], W3ext=weights["W3ext"],
            b1rep=weights["b1rep"], b2rep=weights["b2rep"], b3rep=weights["b3rep"],
            iota=weights["iota"],
            src16=cd["src16"], src32=cd["src32"],
            dcol=cd["dcol"], S2=cd["S2"],
        ))
    return maps


def assemble_output(plan, results):
    C = plan["n_cores"]; NL = plan["NL"]
    outs = [results[c]["out"][:NL] for c in range(C)]
    return np.concatenate(outs, axis=0)


# ---------------- execution harness (PJRT via bass2jax) ----------------
import jax
from jax.sharding import Mesh, PartitionSpec
from jax.experimental.shard_map import shard_map
from concourse.bass2jax import _bass_exec_p, partition_id_tensor, install_neuronx_cc_hook


def build_runner(nc, n_cores):
    install_neuronx_cc_hook()
    partition_name = nc.partition_id_tensor.name if nc.partition_id_tensor else None
    in_names, out_names, out_avals, zero_outs = [], [], [], []
    in_shapes = []
    for alloc in nc.m.functions[0].allocations:
        if not isinstance(alloc, mybir.MemoryLocationSet):
            continue
        name = alloc.memorylocations[0].name
        if alloc.kind == "ExternalInput":
            if name != partition_name and (nc.dbg_addr is None or name != nc.dbg_addr.name):
                in_names.append(name)
                in_shapes.append((tuple(alloc.tensor_shape), mybir.dt.np(alloc.dtype)))
        elif alloc.kind == "ExternalOutput":
            shape = tuple(alloc.tensor_shape)
            dt = mybir.dt.np(alloc.dtype)
            out_names.append(name)
            out_avals.append(jax.core.ShapedArray(shape, dt))
            zero_outs.append(np.zeros(shape, dt))
    n_params = len(in_names)
    n_outs = len(out_names)
    all_in_names = list(in_names) + list(out_names)
    if nc.dbg_addr is not None:
        all_in_names.append(nc.dbg_addr.name)
    if partition_name is not None:
        all_in_names.append(partition_name)

    def _body(*args):
        operands = list(args)
        if nc.dbg_addr is not None:
            operands.append(jax.numpy.zeros((1, 2), jax.numpy.uint32))
        if partition_name is not None:
            operands.append(partition_id_tensor())
        outs = _bass_exec_p.bind(
            *operands,
            out_avals=tuple(out_avals),
            in_names=tuple(all_in_names),
            out_names=tuple(out_names),
            lowering_input_output_aliases=(),
            sim_require_finite=True,
            sim_require_nnan=True,
            nc=nc,
        )
        return tuple(outs)

    devices = jax.devices()[:n_cores]
    mesh = Mesh(np.asarray(devices), ("core",))
    in_specs = (PartitionSpec("core"),) * (n_params + n_outs)
    out_specs = (PartitionSpec("core"),) * n_outs
    sharded = jax.jit(
        shard_map(_body, mesh=mesh, in_specs=in_specs, out_specs=out_specs,
                  check_rep=False),
        keep_unused=True)
    zeros_concat = [np.zeros((n_cores * z.shape[0], *z.shape[1:]), z.dtype)
                    for z in zero_outs]

    from jax.sharding import NamedSharding
    shard = NamedSharding(mesh, PartitionSpec("core"))
    zeros_dev = jax.device_put(zeros_concat, [shard] * len(zeros_concat)) if zeros_concat else []

    in_avals = [jax.ShapeDtypeStruct((n_cores * s[0], *s[1:]), dt, sharding=shard)
                for s, dt in in_shapes]
    out_zero_avals = [jax.ShapeDtypeStruct(z.shape, z.dtype, sharding=shard)
                      for z in zeros_concat]
    compiled = sharded.lower(*in_avals, *out_zero_avals).compile()

    def fn(concat_inputs):
        return compiled(*concat_inputs, *zeros_dev)

    def put(concat_inputs):
        return jax.device_put(concat_inputs, [shard] * len(concat_inputs))

    return fn, in_names, out_names, put, compiled


_CACHE = {}
_LAST = {}


def _get_compiled(plan_key, plan):
    if plan_key not in _CACHE:
        nc = build_program(plan)
        fn, in_names, out_names, put, compiled = build_runner(nc, plan["n_cores"])
        _CACHE[plan_key] = (nc, fn, in_names, out_names, put, compiled)
    return _CACHE[plan_key]


def run_gat(features, weights_kw, src, dst, n_cores=8, n_timing=0):
    n_nodes = features.shape[0]
    plan = host_preprocess(src, dst, n_nodes, n_cores=n_cores, n_chunks=4)
    weights = host_weights(**weights_kw)
    key = (n_nodes, n_cores, bytes(np.asarray(src[:64]).tobytes()),
           plan["n_tiles"])
    nc, fn, in_names, out_names, put, compiled = _get_compiled(key, plan)
    in_maps = make_in_maps(plan, weights, features)
    concat_in = [np.concatenate([np.asarray(in_maps[c][nm])
                                 for c in range(n_cores)], axis=0)
                 for nm in in_names]
    concat_in = put(concat_in)
    _LAST.update(nc=nc, fn=fn, concat_in=concat_in, plan=plan,
                 compiled=compiled, in_names=in_names, out_names=out_names)
    out = fn(concat_in)
    jax.block_until_ready(out)
    times = []
    if n_timing:
        import time
        for _ in range(n_timing):
            t0 = time.perf_counter()
            out = fn(concat_in)
            jax.block_until_ready(out)
            times.append(time.perf_counter() - t0)
    oi = out_names.index("out")
    arr = np.asarray(out[oi])
    NLP = arr.shape[0] // n_cores
    results = [{"out": arr[c * NLP:(c + 1) * NLP]} for c in range(n_cores)]
    full = assemble_output(plan, results)[:n_nodes]
    return full, times


def kernel(features, W1, al1, ar1, b1, W2, al2, ar2, b2, W3, al3, ar3, b3,
           src, dst):
    wk = dict(W1=W1, al1=al1, ar1=ar1, b1=b1, W2=W2, al2=al2, ar2=ar2, b2=b2,
              W3=W3, al3=al3, ar3=ar3, b3=b3)
    out, _ = run_gat(np.asarray(features, np.float32), wk,
                     np.asarray(src), np.asarray(dst), n_cores=8)
    return out.astype(np.float32)


# revision 10
# speedup vs baseline: 11.7544x; 1.0783x over previous
"""GAT Trainium kernel v2: bf16 gathers, on-chip one-hots, matmul-computed er.

v2 changes vs baseline (driven by NTFF profile: GpSimd descriptor generation
for dma_gather was 7.2ms of the 9.16ms exec = ~8ns/index):
- er gathers ELIMINATED (half of all gather indices). Per-edge er is computed
  on the tensor engine: er^T[4k:4k+4, e] = erb_block.T @ S2_tile, where
  S2[d, e] is the (dst x edge) one-hot streamed from host in bf16, and
  erb lives in SBUF [128, NB*heads] (written directly by the node phase - no
  DRAM roundtrip, no dst16 index array at all). One small f32 transpose per
  batch flips er^T back to [e, (t,h)] for the normal-layout score pipeline.
- G rows are 256 bf16 (512B): [h bf16 x128 | el f32 x4 (bitcast) | pad].
  Layer-3 rows are 128 bf16 (256B): [h x40 | el x1 | pad].
- Aggregation one-hot S[e,d] generated ON CHIP per batch:
  S = (dcol[e,t] == iota[d]) on DVE in bf16; host ships dcol [128, NT] bf16
  (pad slots = -1 so they never match).
- All matmuls bf16 (f32 PSUM accumulate); features/weights bf16.
"""
from contextlib import ExitStack
import numpy as np
import ml_dtypes
import concourse.bass as bass
import concourse.tile as tile
from concourse import bacc, mybir
from concourse.masks import make_identity

F32 = mybir.dt.float32
BF16 = mybir.dt.bfloat16
I16 = mybir.dt.int16
BF = ml_dtypes.bfloat16

IN, HID, HEADS, OUT = 256, 32, 4, 40
HH = HID * HEADS  # 128
NEG_SLOPE = 0.2
BT = 16  # tiles per gather batch (16*128 = 2048 idx/call)


def wrap16(a):
    n = a.shape[0]
    assert n % 16 == 0
    blk = a.reshape(-1, 16).T  # [16, n/16]
    return np.tile(blk, (8, 1)).astype(np.int16)


def host_preprocess(src, dst, n_nodes, n_cores=8, n_chunks=4):
    NL = n_nodes // n_cores
    assert NL * n_cores == n_nodes
    NB = (NL + 127) // 128
    CS = (n_nodes + n_chunks - 1) // n_chunks
    assert CS <= 32767

    QR = NL // n_chunks
    assert QR * n_chunks == NL
    src = np.asarray(src); dst = np.asarray(dst)
    core_of = dst // NL
    per_core = []
    counts = np.zeros((n_cores, NB, n_chunks), np.int64)
    for c in range(n_cores):
        m = core_of == c
        s, d = src[m], dst[m]
        dloc = d - c * NL
        # chunk q holds quarter q of every core's slab: node (cs, i) lives at
        # chunk i//QR, row cs*QR + i%QR  (so one quarter-slab AllGather
        # completes one whole chunk)
        cs_ = s // NL
        i_ = s % NL
        ch = i_ // QR
        sloc = cs_ * QR + (i_ % QR)
        order = np.lexsort((dloc, ch))
        dloc, ch, sloc = dloc[order], ch[order], sloc[order]
        b = dloc // 128
        per_core.append((sloc, dloc, ch, b))
        for chh in range(n_chunks):
            mm = ch == chh
            bb, cnt = np.unique(b[mm], return_counts=True)
            counts[c, bb, chh] = cnt
    T = np.ceil(counts.max(axis=0) / 128).astype(np.int64)  # [NB, n_chunks]

    segs = []  # chunk-major: (chunk, block, tiles)
    for chh in range(n_chunks):
        for b in range(NB):
            if T[b, chh] > 0:
                segs.append((chh, b, int(T[b, chh])))
    n_tiles = sum(t for _, _, t in segs)
    total_slots = n_tiles * 128

    batches = []
    cur = None
    tglob = 0
    for chh, b, t in segs:
        for _ in range(t):
            if cur is None or cur["chunk"] != chh or cur["nt"] >= BT:
                if cur is not None:
                    batches.append(cur)
                cur = {"chunk": chh, "t0": tglob, "nt": 0}
            cur["nt"] += 1
            tglob += 1
    if cur is not None:
        batches.append(cur)
    assert tglob == n_tiles

    core_data = []
    for c in range(n_cores):
        sloc, dloc, ch, b = per_core[c]
        src16 = np.zeros(total_slots, np.int16)
        dcol = np.full((128, n_tiles), -1.0, np.float32)  # pad never matches
        S2 = np.zeros((128, total_slots), BF)  # [d, t*128+e] one-hot
        pos = 0
        for chh, bb, t in segs:
            m = (ch == chh) & (b == bb)
            idx = np.nonzero(m)[0]
            n = len(idx)
            cap = t * 128
            assert n <= cap, (c, chh, bb, n, cap)
            sl = sloc[idx]; dl = dloc[idx]
            src16[pos:pos + n] = sl
            e_in_seg = np.arange(n)
            tt = pos // 128 + e_in_seg // 128
            ee = e_in_seg % 128
            dloc_in_b = dl - bb * 128
            dcol[ee, tt] = dloc_in_b
            S2[dloc_in_b, tt * 128 + ee] = 1.0
            pos += cap
        assert pos == total_slots
        core_data.append(dict(
            src16=wrap16(src16),
            dcol=dcol.astype(BF),
            S2=S2,
        ))

    return dict(
        n_cores=n_cores, n_nodes=n_nodes, NL=NL, NB=NB, CS=CS, QR=QR,
        n_chunks=n_chunks, segs=segs, batches=batches, n_tiles=n_tiles,
        total_slots=total_slots, core_data=core_data,
    )


def host_weights(W1, al1, ar1, b1, W2, al2, ar2, b2, W3, al3, ar3, b3):
    def bd(al):
        al = np.asarray(al, np.float32)
        H, F = al.shape
        out = np.zeros((H * F, H), np.float32)
        for h in range(H):
            out[h * F:(h + 1) * F, h] = al[h]
        return out
    W1 = np.asarray(W1, np.float32); W2 = np.asarray(W2, np.float32); W3 = np.asarray(W3, np.float32)
    W1ext = np.concatenate([W1, W1 @ bd(al1), W1 @ bd(ar1)], axis=1)
    W2ext = np.concatenate([W2, W2 @ bd(al2), W2 @ bd(ar2)], axis=1)
    W3ext = np.concatenate([W3, W3 @ bd(al3), W3 @ bd(ar3)], axis=1)
    b1rep = np.tile(np.asarray(b1, np.float32).reshape(1, HH), (128, 1))
    b2rep = np.tile(np.asarray(b2, np.float32).reshape(1, HH), (128, 1))
    b3rep = np.tile(np.asarray(b3, np.float32).reshape(1, OUT), (128, 1))
    iota = np.tile(np.arange(128, dtype=np.float32)[None, :], (128, 1))
    return dict(W1ext=W1ext.astype(BF), W2ext=W2ext.astype(BF),
                W3ext=W3ext.astype(BF),
                b1rep=b1rep, b2rep=b2rep, b3rep=b3rep,
                iota=iota.astype(BF))


def apx(base_ap, col_off, dims):
    """AP at column offset of a [128, W] tile with custom free dims."""
    b = base_ap[:, col_off:col_off + 1]
    return bass.AP(b.tensor, b.offset, [b.ap[0]] + [list(d) for d in dims])


def build_program(plan, stage=99):
    C = plan["n_cores"]; NL = plan["NL"]; NB = plan["NB"]
    NT = plan["n_tiles"]; TS = plan["total_slots"]
    NLP = NB * 128

    nc = bacc.Bacc("TRN2", target_bir_lowering=False, debug=False, num_devices=C)

    featT = nc.dram_tensor("featT", [IN, NL], BF16, kind="ExternalInput").ap()
    W1e = nc.dram_tensor("W1ext", [IN, 136], BF16, kind="ExternalInput").ap()
    W2e = nc.dram_tensor("W2ext", [HH, 136], BF16, kind="ExternalInput").ap()
    W3e = nc.dram_tensor("W3ext", [HH, 42], BF16, kind="ExternalInput").ap()
    B1 = nc.dram_tensor("b1rep", [128, HH], F32, kind="ExternalInput").ap()
    B2 = nc.dram_tensor("b2rep", [128, HH], F32, kind="ExternalInput").ap()
    B3 = nc.dram_tensor("b3rep", [128, OUT], F32, kind="ExternalInput").ap()
    SRC = nc.dram_tensor("src16", [128, TS // 16], I16, kind="ExternalInput").ap()
    DCOL = nc.dram_tensor("dcol", [128, NT], BF16, kind="ExternalInput").ap()
    IOTA = nc.dram_tensor("iota", [128, 128], BF16, kind="ExternalInput").ap()
    S2D = nc.dram_tensor("S2", [128, TS], BF16, kind="ExternalInput").ap()
    OUTT = nc.dram_tensor("out", [NLP, OUT], F32, kind="ExternalOutput").ap()

    G1s = nc.dram_tensor("G1slab", [NL, 256], BF16).ap()
    G2s = nc.dram_tensor("G2slab", [NL, 256], BF16).ap()
    G3s = nc.dram_tensor("G3slab", [NL, 128], BF16).ap()
    G1 = nc.dram_tensor("G1", [C * NL, 256], BF16, addr_space="Shared").ap()
    G2 = nc.dram_tensor("G2", [C * NL, 256], BF16, addr_space="Shared").ap()
    G3 = nc.dram_tensor("G3", [C * NL, 128], BF16, addr_space="Shared").ap()

    rg = [list(range(C))]

    QR = plan["QR"]; NQ = plan["n_chunks"]; CSz = plan["CS"]

    def allgather(slab, full):
        # quarter-interleaved: AG of slab rows [q*QR,(q+1)*QR) from all cores
        # completes gather-chunk q of `full` -> edge phase chunk q can start
        # while later quarters are still gathering
        for q in range(NQ):
            nc.gpsimd.collective_compute(
                "AllGather", mybir.AluOpType.bypass,
                replica_groups=rg,
                ins=[slab[q * QR:(q + 1) * QR, :]],
                outs=[full[q * CSz:(q + 1) * CSz, :]])

    with tile.TileContext(nc) as tc, ExitStack() as ctx:
        const = ctx.enter_context(tc.tile_pool(name="const", bufs=1))
        accp = ctx.enter_context(tc.tile_pool(name="acc", bufs=1))

        w1sb = const.tile([128, 2 * 136], BF16)
        nc.sync.dma_start(w1sb[:, 0:136], W1e[0:128, :])
        nc.sync.dma_start(w1sb[:, 136:272], W1e[128:256, :])
        w2sb = const.tile([128, 136], BF16)
        nc.sync.dma_start(w2sb[:], W2e[:, :])
        w3sb = const.tile([128, 42], BF16)
        nc.sync.dma_start(w3sb[:], W3e[:, :])
        b1sb = const.tile([128, HH], F32)
        nc.sync.dma_start(b1sb[:], B1[:, :])
        b2sb = const.tile([128, HH], F32)
        nc.sync.dma_start(b2sb[:], B2[:, :])
        b3sb = const.tile([128, OUT], F32)
        nc.sync.dma_start(b3sb[:], B3[:, :])
        ident = const.tile([128, 128], F32)
        make_identity(nc, ident[:])
        iotasb = const.tile([128, 128], BF16)
        nc.sync.dma_start(iotasb[:], IOTA[:, :])
        dcolsb = const.tile([128, NT], BF16)
        nc.sync.dma_start(dcolsb[:], DCOL[:, :])
        srcsb = const.tile([128, TS // 16], I16)
        nc.sync.dma_start(srcsb[:], SRC[:, :])
        # SBUF er tables, one per layer: [128 (node-in-block), NB*heads]
        er1sb = const.tile([128, NB * 4], BF16)
        er2sb = const.tile([128, NB * 4], BF16)
        er3sb = const.tile([128, NB], BF16)

        def write_node_rows(lp, ps, nr, b, r0, r1, Gn_s, ersb, n_h, n_el):
            """ps [nr, n_h+n_el+n_er] f32 -> G slab row bf16 + SBUF er table."""
            ncols = Gn_s.shape[1]
            gsb = lp.tile([128, ncols], BF16, tag="gsb")
            nc.vector.tensor_copy(gsb[:nr, 0:n_h], ps[:nr, 0:n_h])
            if n_el == 4:  # f32 el bitcast at bf16 cols [n_h : n_h+8)
                gf = gsb[:].bitcast(F32)
                nc.vector.tensor_copy(gf[:nr, n_h // 2:n_h // 2 + 4],
                                      ps[:nr, n_h:n_h + 4])
                nwr = n_h + 8
            else:  # bf16 el right after h
                nc.vector.tensor_copy(gsb[:nr, n_h:n_h + n_el],
                                      ps[:nr, n_h:n_h + n_el])
                nwr = n_h + n_el
            nc.sync.dma_start(Gn_s[r0:r1, 0:nwr], gsb[:nr, 0:nwr])
            ner = n_el  # heads
            nc.vector.tensor_copy(ersb[:nr, b * ner:(b + 1) * ner],
                                  ps[:nr, n_h + n_el:n_h + n_el + ner])

        # Layer 1 node phase
        with tc.tile_pool(name="l1n", bufs=3) as lp, \
             tc.tile_pool(name="l1np", bufs=2, space="PSUM") as pp:
            for b in range(NB):
                r0 = b * 128
                r1 = min(r0 + 128, NL)
                nr = r1 - r0
                xt = lp.tile([128, 256], BF16, tag="xt")
                nc.sync.dma_start(xt[:, 0:nr], featT[0:128, r0:r1])
                nc.sync.dma_start(xt[:, 128:128 + nr], featT[128:256, r0:r1])
                ps = pp.tile([128, 136], F32, tag="ps")
                nc.tensor.matmul(ps[:nr, :], xt[:, 0:nr], w1sb[:, 0:136],
                                 start=True, stop=False)
                nc.tensor.matmul(ps[:nr, :], xt[:, 128:128 + nr], w1sb[:, 136:272],
                                 start=False, stop=True)
                write_node_rows(lp, ps, nr, b, r0, r1, G1s, er1sb, 128, 4)

        if stage >= 2:
            allgather(G1s, G1)
        if stage >= 3:
            edge_layer(tc, plan, 1, G1, er1sb, srcsb, dcolsb, iotasb, S2D, accp,
                       w_next=w2sb, b_rep=b1sb, Gn_s=G2s, ersb_n=er2sb,
                       ident=ident, write_node=write_node_rows,
                       node_phase=(stage >= 4))
        if stage >= 5:
            allgather(G2s, G2)
            edge_layer(tc, plan, 2, G2, er2sb, srcsb, dcolsb, iotasb, S2D, accp,
                       w_next=w3sb, b_rep=b2sb, Gn_s=G3s, ersb_n=er3sb,
                       ident=ident, write_node=write_node_rows)
            allgather(G3s, G3)
        if stage >= 6:
            edge_layer3(tc, plan, G3, er3sb, srcsb, dcolsb, iotasb, S2D, accp,
                        b3sb, ident, OUTT)

    nc.compile()
    return nc


def seg_maps(plan):
    seg_of_tile = []
    tile_in_seg = []
    for chh, b, t in plan["segs"]:
        seg_of_tile += [(chh, b, t)] * t
        tile_in_seg += list(range(t))
    return seg_of_tile, tile_in_seg


def gen_S(nc, Ssb, dcolsb, iotasb, t0, nt):
    """S[e, k, d] = (dcol[e, t0+k] == iota[d]), bf16, one DVE op."""
    Sv = apx(Ssb[:], 0, [[128, nt], [1, 128]])
    dc = apx(dcolsb[:], t0, [[1, nt], [0, 128]])
    io = apx(iotasb[:], 0, [[0, nt], [1, 128]])
    nc.vector.tensor_tensor(out=Sv, in0=dc, in1=io,
                            op=mybir.AluOpType.is_equal)


def edge_layer(tc, plan, lnum, G, ersb, srcsb, dcolsb, iotasb, S2D, accp,
               w_next, b_rep, Gn_s, ersb_n, ident, write_node,
               node_phase=True):
    nc = tc.nc
    NL = plan["NL"]; NB = plan["NB"]; CS = plan["CS"]
    seg_of_tile, tile_in_seg = seg_maps(plan)

    A = accp.tile([128, NB * 132], F32, tag="A")
    nc.gpsimd.memset(A[:], 0.0)

    with tc.tile_pool(name=f"e{lnum}", bufs=5) as ep, \
         tc.tile_pool(name=f"e{lnum}b", bufs=4) as bp, \
         tc.tile_pool(name=f"e{lnum}s", bufs=4) as sp, \
         tc.tile_pool(name=f"e{lnum}p", bufs=5, space="PSUM") as pp, \
         tc.tile_pool(name=f"e{lnum}e", bufs=3, space="PSUM") as pe:
        ps_cur = {}
        for bt in plan["batches"]:
            nt = bt["nt"]; t0 = bt["t0"]; chh = bt["chunk"]
            nidx = nt * 128
            T = ep.tile([128, BT * 256], BF16, tag="T")
            Tv = T[:].rearrange("p (k d) -> p k d", d=256)[:, 0:nt, :]
            nc.gpsimd.dma_gather(
                Tv, G[chh * CS:(chh + 1) * CS, :],
                srcsb[:, t0 * 8:(t0 + nt) * 8],
                nidx, nidx, 256, single_packet=False)
            # S2 one-hot [d, e] stream for er matmuls
            S2sb = sp.tile([128, BT * 128], BF16, tag="S2")
            nc.sync.dma_start(S2sb[:, 0:nt * 128],
                              S2D[:, t0 * 128:(t0 + nt) * 128])
            # er[e, (k,h)] = S2_tile.T @ erb_block   (per tile; lands in the
            # score layout directly - no transpose needed)
            erps = pe.tile([128, BT * 4], F32, tag="erps")
            for k in range(nt):
                _, b, _ = seg_of_tile[t0 + k]
                nc.tensor.matmul(
                    erps[:, k * 4:(k + 1) * 4],
                    S2sb[:, k * 128:(k + 1) * 128],
                    ersb[:, b * 4:(b + 1) * 4],
                    start=True, stop=True)
            # score = el + er ; el is f32 bitcast at f32-cols [64:68) of T rows
            Tf = T[:].bitcast(F32)
            elv = apx(Tf, 64, [[128, nt], [1, 4]])
            erv = erps[:].rearrange("p (k d) -> p k d", d=4)[:, 0:nt, :]
            sc = ep.tile([128, BT * 4], F32, tag="sc", name="sc")
            scv = sc[:].rearrange("p (k d) -> p k d", d=4)[:, 0:nt, :]
            nc.vector.tensor_tensor(out=scv, in0=elv, in1=erv,
                                    op=mybir.AluOpType.add)
            nc.vector.scalar_tensor_tensor(
                out=scv, in0=scv, scalar=NEG_SLOPE, in1=scv,
                op0=mybir.AluOpType.mult, op1=mybir.AluOpType.max)
            B = bp.tile([128, BT * 132], BF16, tag="B")
            Bv = B[:].rearrange("p (k d) -> p k d", d=132)[:, 0:nt, :]
            nc.scalar.activation(Bv[:, :, 128:132], scv,
                                 mybir.ActivationFunctionType.Exp)
            hw = apx(B[:], 0, [[132, nt], [32, 4], [1, 32]])
            hi = apx(T[:], 0, [[256, nt], [32, 4], [1, 32]])
            ex4 = apx(B[:], 128, [[132, nt], [1, 4], [0, 32]])
            nc.vector.tensor_tensor(out=hw, in0=hi, in1=ex4,
                                    op=mybir.AluOpType.mult)
            Ssb = sp.tile([128, BT * 128], BF16, tag="S")
            gen_S(nc, Ssb, dcolsb, iotasb, t0, nt)
            for k in range(nt):
                tg = t0 + k
                _, b, tseg = seg_of_tile[tg]
                tin = tile_in_seg[tg]
                if tin == 0:
                    ps_cur[b] = pp.tile([128, 132], F32, tag="ps", name="ps")
                ps = ps_cur[b]
                nc.tensor.matmul(
                    ps[:], Ssb[:, k * 128:(k + 1) * 128],
                    B[:, k * 132:(k + 1) * 132],
                    start=(tin == 0), stop=(tin == tseg - 1))
                if tin == tseg - 1:
                    nc.vector.tensor_tensor(
                        out=A[:, b * 132:(b + 1) * 132],
                        in0=A[:, b * 132:(b + 1) * 132],
                        in1=ps[:], op=mybir.AluOpType.add)
                    del ps_cur[b]
        assert not ps_cur

    if not node_phase:
        return
    NOUT = w_next.shape[1]
    n_h = {136: 128, 42: 40}[NOUT]
    n_el = {136: 4, 42: 1}[NOUT]
    with tc.tile_pool(name=f"n{lnum}", bufs=3) as np_, \
         tc.tile_pool(name=f"n{lnum}p", bufs=4, space="PSUM") as pp:
        for b in range(NB):
            r0 = b * 128
            r1 = min(r0 + 128, NL)
            nr = r1 - r0
            Ab = A[:, b * 132:(b + 1) * 132]
            rs = np_.tile([128, 4], F32, tag="rs")
            nc.vector.tensor_scalar_max(rs[:], Ab[:, 128:132], 1e-30)
            nc.vector.reciprocal(rs[:], rs[:])
            hp = np_.tile([128, 128], F32, tag="hp")
            hv = hp[:].rearrange("p (g f) -> p g f", g=4)
            rsb = apx(rs[:], 0, [[1, 4], [0, 32]])
            av = Ab[:, 0:128].rearrange("p (g f) -> p g f", g=4)
            nc.vector.tensor_tensor(out=hv, in0=av, in1=rsb,
                                    op=mybir.AluOpType.mult)
            nc.vector.tensor_tensor(out=hp[:], in0=hp[:], in1=b_rep[:],
                                    op=mybir.AluOpType.add)
            nc.scalar.activation(hp[:], hp[:], mybir.ActivationFunctionType.Relu)
            pst = pp.tile([128, 128], F32, tag="pst")
            nc.tensor.transpose(out=pst[:], in_=hp[:], identity=ident[:])
            hpt = np_.tile([128, 128], BF16, tag="hpt")
            nc.vector.tensor_copy(hpt[:], pst[:])
            ps2 = pp.tile([128, NOUT], F32, tag="ps2")
            nc.tensor.matmul(ps2[:nr, :], hpt[:, 0:nr], w_next[:],
                             start=True, stop=True)
            write_node(np_, ps2, nr, b, r0, r1, Gn_s, ersb_n, n_h, n_el)


def edge_layer3(tc, plan, G, ersb, srcsb, dcolsb, iotasb, S2D, accp,
                b3sb, ident, OUTT):
    nc = tc.nc
    NL = plan["NL"]; NB = plan["NB"]; CS = plan["CS"]
    seg_of_tile, tile_in_seg = seg_maps(plan)

    A = accp.tile([128, NB * 132], F32, tag="A")
    Av = A[:, 0:NB * 41]
    nc.gpsimd.memset(A[:], 0.0)

    with tc.tile_pool(name="e3", bufs=5) as ep, \
         tc.tile_pool(name="e3b", bufs=4) as bp, \
         tc.tile_pool(name="e3s", bufs=4) as sp, \
         tc.tile_pool(name="e3p", bufs=5, space="PSUM") as pp, \
         tc.tile_pool(name="e3e", bufs=3, space="PSUM") as pe:
        ps_cur = {}
        for bt in plan["batches"]:
            nt = bt["nt"]; t0 = bt["t0"]; chh = bt["chunk"]
            nidx = nt * 128
            T = ep.tile([128, BT * 128], BF16, tag="T3")
            Tv = T[:].rearrange("p (k d) -> p k d", d=128)[:, 0:nt, :]
            nc.gpsimd.dma_gather(
                Tv, G[chh * CS:(chh + 1) * CS, :],
                srcsb[:, t0 * 8:(t0 + nt) * 8],
                nidx, nidx, 128, single_packet=False)
            S2sb = sp.tile([128, BT * 128], BF16, tag="S23")
            nc.sync.dma_start(S2sb[:, 0:nt * 128],
                              S2D[:, t0 * 128:(t0 + nt) * 128])
            erps = pe.tile([128, BT], F32, tag="erps3")
            for k in range(nt):
                _, b, _ = seg_of_tile[t0 + k]
                nc.tensor.matmul(
                    erps[:, k:k + 1],
                    S2sb[:, k * 128:(k + 1) * 128],
                    ersb[:, b:b + 1],
                    start=True, stop=True)
            sc = ep.tile([128, BT], F32, tag="sc3", name="sc3")
            scv = sc[:].rearrange("p (k d) -> p k d", d=1)[:, 0:nt, :]
            erv = erps[:].rearrange("p (k d) -> p k d", d=1)[:, 0:nt, :]
            nc.vector.tensor_tensor(out=scv, in0=Tv[:, :, 40:41], in1=erv,
                                    op=mybir.AluOpType.add)
            nc.vector.scalar_tensor_tensor(
                out=scv, in0=scv, scalar=NEG_SLOPE, in1=scv,
                op0=mybir.AluOpType.mult, op1=mybir.AluOpType.max)
            B = bp.tile([128, BT * 41], BF16, tag="B3")
            Bv = B[:].rearrange("p (k d) -> p k d", d=41)[:, 0:nt, :]
            nc.scalar.activation(Bv[:, :, 40:41], scv,
                                 mybir.ActivationFunctionType.Exp)
            hw = apx(B[:], 0, [[41, nt], [1, 40]])
            hi = apx(T[:], 0, [[128, nt], [1, 40]])
            ex1 = apx(B[:], 40, [[41, nt], [0, 40]])
            nc.vector.tensor_tensor(out=hw, in0=hi, in1=ex1,
                                    op=mybir.AluOpType.mult)
            Ssb = sp.tile([128, BT * 128], BF16, tag="S3")
            gen_S(nc, Ssb, dcolsb, iotasb, t0, nt)
            for k in range(nt):
                tg = t0 + k
                _, b, tseg = seg_of_tile[tg]
                tin = tile_in_seg[tg]
                if tin == 0:
                    ps_cur[b] = pp.tile([128, 41], F32, tag="ps3", name="ps3")
                ps = ps_cur[b]
                nc.tensor.matmul(
                    ps[:], Ssb[:, k * 128:(k + 1) * 128],
                    B[:, k * 41:(k + 1) * 41],
                    start=(tin == 0), stop=(tin == tseg - 1))
                if tin == tseg - 1:
                    nc.vector.tensor_tensor(
                        out=Av[:, b * 41:(b + 1) * 41],
                        in0=Av[:, b * 41:(b + 1) * 41],
                        in1=ps[:], op=mybir.AluOpType.add)
                    del ps_cur[b]
        assert not ps_cur

    with tc.tile_pool(name="n3", bufs=1) as np_:
        O = np_.tile([128, NB * 40], F32, tag="O")
        for b in range(NB):
            Ab = Av[:, b * 41:(b + 1) * 41]
            rs = np_.tile([128, 1], F32, tag="rs3")
            nc.vector.tensor_scalar_max(rs[:], Ab[:, 40:41], 1e-30)
            nc.vector.reciprocal(rs[:], rs[:])
            rsb = apx(rs[:], 0, [[0, 40]])
            Ob = O[:, b * 40:(b + 1) * 40]
            nc.vector.tensor_tensor(out=Ob, in0=Ab[:, 0:40], in1=rsb,
                                    op=mybir.AluOpType.mult)
            nc.vector.tensor_tensor(out=Ob, in0=Ob, in1=b3sb[:],
                                    op=mybir.AluOpType.add)
        Ovv = O[:].rearrange("p (b f) -> p b f", f=40)
        mx = np_.tile([128, NB], F32, tag="mx")
        nc.vector.tensor_reduce(mx[:], Ovv, axis=mybir.AxisListType.X,
                                op=mybir.AluOpType.max)
        mxb = apx(mx[:], 0, [[1, NB], [0, 40]])
        nc.vector.tensor_tensor(out=Ovv, in0=Ovv, in1=mxb,
                                op=mybir.AluOpType.subtract)
        E = np_.tile([128, NB * 40], F32, tag="E")
        nc.scalar.activation(E[:], O[:], mybir.ActivationFunctionType.Exp)
        ss = np_.tile([128, NB], F32, tag="ss")
        nc.vector.tensor_reduce(ss[:], E[:].rearrange("p (b f) -> p b f", f=40),
                                axis=mybir.AxisListType.X, op=mybir.AluOpType.add)
        nc.scalar.activation(ss[:], ss[:], mybir.ActivationFunctionType.Ln)
        ssb = apx(ss[:], 0, [[1, NB], [0, 40]])
        nc.vector.tensor_tensor(out=Ovv, in0=Ovv, in1=ssb,
                                op=mybir.AluOpType.subtract)
        nc.sync.dma_start(OUTT[:, :].rearrange("(b p) f -> p b f", p=128), Ovv)


def make_in_maps(plan, weights, features):
    """Per-core input dicts."""
    C = plan["n_cores"]; NL = plan["NL"]
    features = np.asarray(features, np.float32).astype(BF)
    maps = []
    for c in range(C):
        cd = plan["core_data"][c]
        maps.append(dict(
            featT=np.ascontiguousarray(features[c * NL:(c + 1) * NL].T),
            W1ext=weights["W1ext"], W2ext=weights["W2ext"], W3ext=weights["W3ext"],
            b1rep=weights["b1rep"], b2rep=weights["b2rep"], b3rep=weights["b3rep"],
            iota=weights["iota"],
            src16=cd["src16"], dcol=cd["dcol"], S2=cd["S2"],
        ))
    return maps


def assemble_output(plan, results):
    C = plan["n_cores"]; NL = plan["NL"]
    outs = [results[c]["out"][:NL] for c in range(C)]
    return np.concatenate(outs, axis=0)


# ---------------- execution harness (PJRT via bass2jax) ----------------
import jax
from jax.sharding import Mesh, PartitionSpec
from jax.experimental.shard_map import shard_map
from concourse.bass2jax import _bass_exec_p, partition_id_tensor, install_neuronx_cc_hook


def build_runner(nc, n_cores):
    install_neuronx_cc_hook()
    partition_name = nc.partition_id_tensor.name if nc.partition_id_tensor else None
    in_names, out_names, out_avals, zero_outs = [], [], [], []
    in_shapes = []
    for alloc in nc.m.functions[0].allocations:
        if not isinstance(alloc, mybir.MemoryLocationSet):
            continue
        name = alloc.memorylocations[0].name
        if alloc.kind == "ExternalInput":
            if name != partition_name and (nc.dbg_addr is None or name != nc.dbg_addr.name):
                in_names.append(name)
                in_shapes.append((tuple(alloc.tensor_shape), mybir.dt.np(alloc.dtype)))
        elif alloc.kind == "ExternalOutput":
            shape = tuple(alloc.tensor_shape)
            dt = mybir.dt.np(alloc.dtype)
            out_names.append(name)
            out_avals.append(jax.core.ShapedArray(shape, dt))
            zero_outs.append(np.zeros(shape, dt))
    n_params = len(in_names)
    n_outs = len(out_names)
    all_in_names = list(in_names) + list(out_names)
    if nc.dbg_addr is not None:
        all_in_names.append(nc.dbg_addr.name)
    if partition_name is not None:
        all_in_names.append(partition_name)

    def _body(*args):
        operands = list(args)
        if nc.dbg_addr is not None:
            operands.append(jax.numpy.zeros((1, 2), jax.numpy.uint32))
        if partition_name is not None:
            operands.append(partition_id_tensor())
        outs = _bass_exec_p.bind(
            *operands,
            out_avals=tuple(out_avals),
            in_names=tuple(all_in_names),
            out_names=tuple(out_names),
            lowering_input_output_aliases=(),
            sim_require_finite=True,
            sim_require_nnan=True,
            nc=nc,
        )
        return tuple(outs)

    devices = jax.devices()[:n_cores]
    mesh = Mesh(np.asarray(devices), ("core",))
    in_specs = (PartitionSpec("core"),) * (n_params + n_outs)
    out_specs = (PartitionSpec("core"),) * n_outs
    sharded = jax.jit(
        shard_map(_body, mesh=mesh, in_specs=in_specs, out_specs=out_specs,
                  check_rep=False),
        keep_unused=True)
    zeros_concat = [np.zeros((n_cores * z.shape[0], *z.shape[1:]), z.dtype)
                    for z in zero_outs]

    from jax.sharding import NamedSharding
    shard = NamedSharding(mesh, PartitionSpec("core"))
    zeros_dev = jax.device_put(zeros_concat, [shard] * len(zeros_concat)) if zeros_concat else []

    in_avals = [jax.ShapeDtypeStruct((n_cores * s[0], *s[1:]), dt, sharding=shard)
                for s, dt in in_shapes]
    out_zero_avals = [jax.ShapeDtypeStruct(z.shape, z.dtype, sharding=shard)
                      for z in zeros_concat]
    compiled = sharded.lower(*in_avals, *out_zero_avals).compile()

    def fn(concat_inputs):
        return compiled(*concat_inputs, *zeros_dev)

    def put(concat_inputs):
        return jax.device_put(concat_inputs, [shard] * len(concat_inputs))

    return fn, in_names, out_names, put, compiled


_CACHE = {}
_LAST = {}


def _get_compiled(plan_key, plan):
    if plan_key not in _CACHE:
        nc = build_program(plan)
        fn, in_names, out_names, put, compiled = build_runner(nc, plan["n_cores"])
        _CACHE[plan_key] = (nc, fn, in_names, out_names, put, compiled)
    return _CACHE[plan_key]


def run_gat(features, weights_kw, src, dst, n_cores=8, n_timing=0):
    n_nodes = features.shape[0]
    plan = host_preprocess(src, dst, n_nodes, n_cores=n_cores, n_chunks=4)
    weights = host_weights(**weights_kw)
    key = (n_nodes, n_cores, bytes(np.asarray(src[:64]).tobytes()),
           plan["n_tiles"])
    nc, fn, in_names, out_names, put, compiled = _get_compiled(key, plan)
    in_maps = make_in_maps(plan, weights, features)
    concat_in = [np.concatenate([np.asarray(in_maps[c][nm])
                                 for c in range(n_cores)], axis=0)
                 for nm in in_names]
    concat_in = put(concat_in)
    _LAST.update(nc=nc, fn=fn, concat_in=concat_in, plan=plan,
                 compiled=compiled, in_names=in_names, out_names=out_names)
    out = fn(concat_in)
    jax.block_until_ready(out)
    times = []
    if n_timing:
        import time
        for _ in range(n_timing):
            t0 = time.perf_counter()
            out = fn(concat_in)
            jax.block_until_ready(out)
            times.append(time.perf_counter() - t0)
    oi = out_names.index("out")
    arr = np.asarray(out[oi])
    NLP = arr.shape[0] // n_cores
    results = [{"out": arr[c * NLP:(c + 1) * NLP]} for c in range(n_cores)]
    full = assemble_output(plan, results)[:n_nodes]
    return full, times


def kernel(features, W1, al1, ar1, b1, W2, al2, ar2, b2, W3, al3, ar3, b3,
           src, dst):
    wk = dict(W1=W1, al1=al1, ar1=ar1, b1=b1, W2=W2, al2=al2, ar2=ar2, b2=b2,
              W3=W3, al3=al3, ar3=ar3, b3=b3)
    out, _ = run_gat(np.asarray(features, np.float32), wk,
                     np.asarray(src), np.asarray(dst), n_cores=8)
    return out.astype(np.float32)
